# revision 1
# baseline (speedup 1.0000x reference)
"""DAHead (dual-attention head) Trainium2 kernel.

8-core SPMD: core c handles sample c//2, spatial half c%2.
The half-split uses a flip trick so every core runs the SAME program:
odd cores receive the sample vertically flipped (and conv weights
flipped along dy); conv/attention/upsample all commute with the flip,
and the host un-flips the output half.

Per-core program:
  1. conv3x3+BN+lrelu (PAM) in bf16x2 (3-term hi/lo split - near-fp32,
     needed because the softmax downstream is argmax-like), features
     kept as bf16 hi/lo pairs.
  2. q/k (fp32 evac of bf16x2 matmuls) and v^T (bf16x2) projections,
     spilled to DRAM scratch (keeps SBUF pool lifetimes nested).
  3. Attention over own i-range (local rows 0..33): logits in plain
     fp32, softmax (ACT exp + DVE), PE transpose of the prob rows,
     apply in fp32r, residual, 1x1 out-proj in bf16x2.
  4. conv3x3+BN+lrelu (CAM) in fp32r, channel attention (mean -> MLP ->
     sigmoid, scale folded into the 1x1 weights), 1x1 out-proj fp32r,
     accumulated into the PAM output.
  5. bilinear x2 upsample (DVE) of the own half, DMA out.
"""
import sys

if '/opt/trn_rl_repo' not in sys.path:
    sys.path.insert(0, '/opt/trn_rl_repo')

import numpy as np
import ml_dtypes

import concourse.bass as bass
import concourse.mybir as mybir
import concourse.tile as tile
from concourse import bacc
from concourse.bass_utils import run_bass_kernel_spmd

dt = mybir.dt
f32 = dt.float32
f32r = dt.float32r
bf16 = dt.bfloat16
fp16 = dt.float16
BF = ml_dtypes.bfloat16
AF = mybir.ActivationFunctionType
OP = mybir.AluOpType

C = 512          # channels
P = 128          # partition size
NCH = C // P     # channel chunks (4)
H = W = 64
HW = H * W       # 4096
CR = 64          # q/k channels
OC = 64          # output channels
OWN_ROWS = 34    # local rows handled per core (rows 0..33)
OWN = OWN_ROWS * W    # 2176 = 17*128
NIC = OWN // P        # 17 attention i-chunks
NJC = HW // P         # 32 j-chunks
HB = 8                # conv h-blocks of 8 rows
EPS = 1e-5

# tap order: full-coverage center tap first (needed for PSUM start flag)
_ALL = [(ci, dy, dx) for ci in range(NCH) for dy in (-1, 0, 1) for dx in (-1, 0, 1)]
TAPS = [(0, 0, 0)] + [t for t in _ALL if t != (0, 0, 0)]
NT = len(TAPS)   # 36

# block edges for reads of the (OWN | rest) split f store
K_EDGES = [0, 512, 1024, 1536, 2048, OWN, OWN + 512, OWN + 1024, OWN + 1424, HW]
Q_EDGES = [0, 512, 1024, 1536, 2048, OWN]


def _conv_tap_aps(psum_t, x_t, hb, dy, dx):
    """APs for one conv tap on h-block hb. psum_t: [128,8,64], x_t: [128,64,64]."""
    r0 = 1 if (hb == 0 and dy == -1) else 0
    r1 = 7 if (hb == HB - 1 and dy == 1) else 8
    c0 = 1 if dx == -1 else 0
    c1 = 63 if dx == 1 else 64
    out_ap = psum_t[:, r0:r1, c0:c1]
    in_ap = x_t[:, hb * 8 + r0 + dy: hb * 8 + r1 + dy, c0 + dx: c1 + dx]
    return out_ap, in_ap


def _emit_phase1(nc, tc, d, ct, f_store):
    """PAM conv3x3 + BN + lrelu in bf16x2."""
    with tc.tile_pool(name="xs_pam", bufs=1) as px, \
         tc.tile_pool(name="wpam", bufs=3) as pw, \
         tc.tile_pool(name="pam_evac", bufs=3) as pe, \
         tc.tile_pool(name="ps_conv", bufs=1, space="PSUM") as psc:
        xh_t = [px.tile([P, H, W], bf16, name=f"xh{i}", tag=f"xh{i}") for i in range(NCH)]
        xl_t = [px.tile([P, H, W], bf16, name=f"xl{i}", tag=f"xl{i}") for i in range(NCH)]
        for i in range(NCH):
            nc.sync.dma_start(out=xh_t[i], in_=d['xh'][i])
            nc.sync.dma_start(out=xl_t[i], in_=d['xl'][i])
        for co in range(NCH):
            wh = pw.tile([P, NT, P], bf16, tag="w", name="wh")
            wl = pw.tile([P, NT, P], bf16, tag="w", name="wl")
            nc.sync.dma_start(out=wh, in_=d['wph'][co].rearrange("t p f -> p t f"))
            nc.sync.dma_start(out=wl, in_=d['wpl'][co].rearrange("t p f -> p t f"))
            pst = [psc.tile([P, 8, W], f32, tag=f"cv{hb}", name=f"cv{hb}")
                   for hb in range(HB)]
            for t, (ci, dy, dx) in enumerate(TAPS):
                for term in range(3):
                    wt = wh if term < 2 else wl
                    xt = xh_t if term != 1 else xl_t
                    for hb in range(HB):
                        o_ap, i_ap = _conv_tap_aps(pst[hb], xt[ci], hb, dy, dx)
                        nc.tensor.matmul(
                            o_ap, wt[:, t, :], i_ap,
                            start=(t == 0 and term == 0),
                            stop=(t == NT - 1 and term == 2))
            for hb in range(HB):
                z = pe.tile([P, 8 * W], f32, tag="z", name="z")
                nc.scalar.activation(
                    out=z, in_=pst[hb].rearrange("p a b -> p (a b)"),
                    func=AF.Identity, bias=ct['bp'][co], scale=ct['sp'][co])
                ft = pe.tile([P, 8 * W], f32, tag="ft", name="ft")
                nc.vector.scalar_tensor_tensor(
                    out=ft, in0=z, scalar=0.2, in1=z, op0=OP.mult, op1=OP.max)
                f_store(co, hb * 8 * W, ft)


def _emit_phase2(nc, tc, d, ct, f_read, qsc, ksc, vsc):
    """q/k (bf16x2, fp32 result) and vT (bf16x2) projections -> DRAM scratch."""
    with tc.tile_pool(name="qk_sb", bufs=1) as pqs, \
         tc.tile_pool(name="qk_w", bufs=1) as pqw, \
         tc.tile_pool(name="v_ev", bufs=2) as pve, \
         tc.tile_pool(name="ps_qkv", bufs=2, space="PSUM") as psq:
        q_t = pqs.tile([CR, OWN], f32, name="q_t", tag="q_t")
        k_t = pqs.tile([CR, HW], f32, name="k_t", tag="k_t")
        wq_h = [pqw.tile([P, CR], bf16, name=f"wqh{i}", tag=f"wqh{i}") for i in range(NCH)]
        wq_l = [pqw.tile([P, CR], bf16, name=f"wql{i}", tag=f"wql{i}") for i in range(NCH)]
        wk_h = [pqw.tile([P, CR], bf16, name=f"wkh{i}", tag=f"wkh{i}") for i in range(NCH)]
        wk_l = [pqw.tile([P, CR], bf16, name=f"wkl{i}", tag=f"wkl{i}") for i in range(NCH)]
        wv_h = [pqw.tile([P, C], bf16, name=f"wvh{i}", tag=f"wvh{i}") for i in range(NCH)]
        wv_l = [pqw.tile([P, C], bf16, name=f"wvl{i}", tag=f"wvl{i}") for i in range(NCH)]
        bv_t = pqw.tile([P, C], f32, name="bv_t", tag="bv_t")
        nc.sync.dma_start(out=bv_t, in_=d['bv'].to_broadcast([P, C]))
        for i in range(NCH):
            nc.sync.dma_start(out=wq_h[i], in_=d['wqh'][i])
            nc.sync.dma_start(out=wq_l[i], in_=d['wql'][i])
            nc.sync.dma_start(out=wk_h[i], in_=d['wkh'][i])
            nc.sync.dma_start(out=wk_l[i], in_=d['wkl'][i])
            nc.sync.dma_start(out=wv_h[i], in_=d['wvh'][i])
            nc.sync.dma_start(out=wv_l[i], in_=d['wvl'][i])

        def proj_qk(dst, wts_h, wts_l, bias_t, edges):
            for bi in range(len(edges) - 1):
                off, end = edges[bi], edges[bi + 1]
                sz = end - off
                pq = psq.tile([CR, 512], f32, tag="pq", name="pq")[:, 0:sz]
                first = True
                for ci in range(NCH):
                    # terms: (w_hi,f_hi), (w_hi,f_lo), (w_lo,f_hi)
                    for term in range(3):
                        wt = wts_h[ci] if term < 2 else wts_l[ci]
                        xin = f_read(term != 1, ci, off, end)
                        nc.tensor.matmul(pq, wt, xin, start=first,
                                         stop=(ci == NCH - 1 and term == 2))
                        first = False
                nc.scalar.activation(out=dst[:, off:end], in_=pq,
                                     func=AF.Identity, bias=bias_t, scale=1.0)

        proj_qk(q_t, wq_h, wq_l, ct['bq'], Q_EDGES)
        proj_qk(k_t, wk_h, wk_l, ct['bk'], K_EDGES)
        nc.sync.dma_start(out=qsc, in_=q_t)
        nc.sync.dma_start(out=ksc, in_=k_t)

        for jc in range(NJC):
            pv = psq.tile([P, C], f32, tag="pv", name="pv")
            s, e = jc * P, (jc + 1) * P
            # terms: (f_hi,wv_hi), (f_hi,wv_lo), (f_lo,wv_hi)
            for term in range(3):
                for ci in range(NCH):
                    lhs = f_read(term != 2, ci, s, e)
                    rhs = (wv_l if term == 1 else wv_h)[ci]
                    nc.tensor.matmul(pv, lhs, rhs,
                                     start=(term == 0 and ci == 0),
                                     stop=(term == 2 and ci == NCH - 1))
            vtmp = pve.tile([P, C], fp16, tag="vtmp", name="vtmp")
            nc.vector.tensor_add(vtmp, pv, bv_t)
            nc.sync.dma_start(out=vsc[:, jc, :], in_=vtmp)


def _emit_attention(nc, tc, ct, pam_sb, fsc, qsc, ksc, vsc):
    with tc.tile_pool(name="qk2", bufs=1) as pq2, \
         tc.tile_pool(name="vt2", bufs=1) as pv2, \
         tc.tile_pool(name="ls", bufs=1) as pls, \
         tc.tile_pool(name="et", bufs=1) as pet, \
         tc.tile_pool(name="fstream", bufs=2) as pfs, \
         tc.tile_pool(name="att_tmp", bufs=2) as pat, \
         tc.tile_pool(name="res_t", bufs=2) as prs, \
         tc.tile_pool(name="ps_l", bufs=2, space="PSUM") as psl, \
         tc.tile_pool(name="ps_t", bufs=2, space="PSUM") as pstp, \
         tc.tile_pool(name="ps_a", bufs=2, space="PSUM") as psa, \
         tc.tile_pool(name="ps_p", bufs=1, space="PSUM") as psp:
        q_t = pq2.tile([CR, OWN], f32, name="q2_t", tag="q2_t")
        k_t = pq2.tile([CR, HW], f32, name="k2_t", tag="k2_t")
        vt_t = pv2.tile([P, NJC, C], fp16, name="vt2_t", tag="vt2_t")
        nc.sync.dma_start(out=q_t, in_=qsc)
        nc.sync.dma_start(out=k_t, in_=ksc)
        nc.sync.dma_start(out=vt_t, in_=vsc)
        n_blocks = (NIC + 1) // 2
        for ib in range(n_blocks):
            ics = [2 * ib, 2 * ib + 1]
            if ics[-1] >= NIC:
                ics = ics[:1]
            isz = P * len(ics)
            ioff = ics[0] * P
            et_t = pet.tile([P, NJC, 2 * P], fp16, tag="et", name="et")
            for ph, ic in enumerate(ics):
                ls = pls.tile([P, HW], f32, tag="ls", name="ls")
                for jb in range(HW // 512):
                    pl = psl.tile([P, 512], f32, tag="pl", name="pl")
                    nc.tensor.matmul(
                        pl, q_t[:, ic * P:(ic + 1) * P],
                        k_t[:, jb * 512:(jb + 1) * 512], start=True, stop=True)
                    nc.scalar.activation(
                        out=ls[:, jb * 512:(jb + 1) * 512], in_=pl,
                        func=AF.Identity, bias=0.0, scale=1.0)
                nmax = pat.tile([P, 1], f32, tag="nmax", name="nmax")
                nc.vector.tensor_reduce(out=nmax, in_=ls, axis=mybir.AxisListType.X,
                                        op=OP.max, negate=True)
                rsum = pat.tile([P, 1], f32, tag="rsum", name="rsum")
                nc.scalar.activation(out=ls, in_=ls, func=AF.Exp,
                                     bias=nmax, scale=1.0, accum_out=rsum)
                rrec = pat.tile([P, 1], f32, tag="rrec", name="rrec")
                nc.vector.reciprocal(out=rrec, in_=rsum)
                e16 = pls.tile([P, HW], fp16, tag="e16", name="e16")
                nc.vector.tensor_scalar_mul(e16, ls, rrec)
                for jc in range(NJC):
                    pt = pstp.tile([P, P], fp16, tag="pt", name="pt")
                    nc.tensor.transpose(
                        pt, e16[:, jc * P:(jc + 1) * P], ct['ident'])
                    nc.vector.tensor_copy(
                        out=et_t[:, jc, ph * P:(ph + 1) * P], in_=pt)
            res_h, res_l = [], []
            for co in range(NCH):
                fs_h = pfs.tile([P, 2 * P], bf16, tag=f"fsh{co}",
                                name=f"fsh{co}")[:, 0:isz]
                fs_l = pfs.tile([P, 2 * P], bf16, tag=f"fsl{co}",
                                name=f"fsl{co}")[:, 0:isz]
                nc.sync.dma_start(out=fs_h, in_=fsc[co, 0, :, ioff:ioff + isz])
                nc.sync.dma_start(out=fs_l, in_=fsc[co, 1, :, ioff:ioff + isz])
                pa = psa.tile([P, 2 * P], f32, tag="pa", name="pa")[:, 0:isz]
                for jc in range(NJC):
                    nc.tensor.matmul(
                        pa, vt_t[:, jc, co * P:(co + 1) * P],
                        et_t[:, jc, 0:isz],
                        start=(jc == 0), stop=(jc == NJC - 1))
                rt = prs.tile([P, 2 * P], f32, tag="rt", name="rt")[:, 0:isz]
                nc.vector.scalar_tensor_tensor(
                    out=rt, in0=pa, scalar=ct['alpha'], in1=fs_h,
                    op0=OP.mult, op1=OP.add)
                nc.vector.tensor_add(rt, rt, fs_l)
                rh = prs.tile([P, 2 * P], bf16, tag=f"rh{co}", name=f"rh{co}")[:, 0:isz]
                nc.vector.tensor_copy(out=rh, in_=rt)
                rl = prs.tile([P, 2 * P], bf16, tag=f"rl{co}", name=f"rl{co}")[:, 0:isz]
                nc.vector.tensor_sub(rl, rt, rh)
                res_h.append(rh)
                res_l.append(rl)
            pp = psp.tile([OC, 2 * P], f32, tag="pp", name="pp")[:, 0:isz]
            first = True
            for ci in range(NCH):
                for term in range(3):
                    wt = ct['wpoh'][ci] if term < 2 else ct['wpol'][ci]
                    rs = res_h[ci] if term != 1 else res_l[ci]
                    nc.tensor.matmul(pp, wt, rs, start=first,
                                     stop=(ci == NCH - 1 and term == 2))
                    first = False
            nc.scalar.activation(out=pam_sb[:, ioff:ioff + isz], in_=pp,
                                 func=AF.Identity, bias=ct['bpo'], scale=1.0)


def _emit_cam(nc, tc, d, ct, pam_sb):
    with tc.tile_pool(name="xs_cam", bufs=1) as pxc, \
         tc.tile_pool(name="g_store", bufs=1) as pg:
        x_t = [pxc.tile([P, H, W], fp16, name=f"x{i}", tag=f"x{i}") for i in range(NCH)]
        for i in range(NCH):
            nc.sync.dma_start(out=x_t[i], in_=d['xs'][i])
        g_t = [pg.tile([P, HW], fp16, name=f"g{i}", tag=f"g{i}") for i in range(NCH)]
        with tc.tile_pool(name="wcam", bufs=2) as pwc, \
             tc.tile_pool(name="ps_conv2", bufs=1, space="PSUM") as psc2:
            for co in range(NCH):
                wc = pwc.tile([P, NT, P], fp16, tag="wc", name="wc")
                nc.sync.dma_start(out=wc, in_=d['wcm'][co].rearrange("t p f -> p t f"))
                pst = [psc2.tile([P, 8, W], f32, tag=f"cv{hb}", name=f"cv{hb}")
                       for hb in range(HB)]
                for t, (ci, dy, dx) in enumerate(TAPS):
                    for hb in range(HB):
                        o_ap, i_ap = _conv_tap_aps(pst[hb], x_t[ci], hb, dy, dx)
                        nc.tensor.matmul(
                            o_ap, wc[:, t, :], i_ap,
                            start=(t == 0), stop=(t == NT - 1))
                for hb in range(HB):
                    gsl = g_t[co][:, hb * 8 * W:(hb + 1) * 8 * W]
                    zc2 = pwc.tile([P, 8 * W], f32, tag="zc2", name="zc2")
                    nc.scalar.activation(
                        out=zc2, in_=pst[hb].rearrange("p a b -> p (a b)"),
                        func=AF.Identity, bias=ct['bc'][co], scale=ct['sc'][co])
                    nc.vector.scalar_tensor_tensor(
                        out=gsl, in0=zc2, scalar=0.2, in1=zc2,
                        op0=OP.mult, op1=OP.max)
        # channel attention MLP + 1x1 out
        with tc.tile_pool(name="mlp", bufs=1) as pm, \
             tc.tile_pool(name="cam_ev", bufs=2) as pce, \
             tc.tile_pool(name="ps_mlp", bufs=2, space="PSUM") as psm, \
             tc.tile_pool(name="ps_co", bufs=2, space="PSUM") as psco:
            msum = [pm.tile([P, 1], f32, name=f"ms{i}", tag=f"ms{i}") for i in range(NCH)]
            for i in range(NCH):
                nc.vector.tensor_reduce(out=msum[i], in_=g_t[i],
                                        axis=mybir.AxisListType.X, op=OP.add)
            wc1_t = [pm.tile([P, CR], f32, name=f"w1{i}", tag=f"w1{i}") for i in range(NCH)]
            wc2_t = [pm.tile([CR, P], f32, name=f"w2{i}", tag=f"w2{i}") for i in range(NCH)]
            wco_t = [pm.tile([P, OC], f32, name=f"wo{i}", tag=f"wo{i}") for i in range(NCH)]
            bc2_t = [pm.tile([P, 1], f32, name=f"b2{i}", tag=f"b2{i}") for i in range(NCH)]
            for i in range(NCH):
                nc.sync.dma_start(out=wc1_t[i], in_=d['wc1'][i])
                nc.sync.dma_start(out=wc2_t[i], in_=d['wc2'][i])
                nc.sync.dma_start(out=wco_t[i], in_=d['wco'][i])
                nc.sync.dma_start(out=bc2_t[i], in_=d['bc2'][i])
            p1 = psm.tile([CR, 1], f32, tag="p1", name="p1")
            for ci in range(NCH):
                nc.tensor.matmul(p1, wc1_t[ci], msum[ci],
                                 start=(ci == 0), stop=(ci == NCH - 1))
            t1 = pm.tile([CR, 1], f32, name="t1", tag="t1")
            nc.scalar.activation(out=t1, in_=p1, func=AF.Identity,
                                 bias=ct['bc1'], scale=1.0)
            y1 = pm.tile([CR, 1], f32, name="y1", tag="y1")
            nc.vector.scalar_tensor_tensor(out=y1, in0=t1, scalar=0.2, in1=t1,
                                           op0=OP.mult, op1=OP.max)
            s_t = [pm.tile([P, 1], f32, name=f"s{i}", tag=f"s{i}") for i in range(NCH)]
            wce = [pm.tile([P, OC], fp16, name=f"we{i}", tag=f"we{i}") for i in range(NCH)]
            for co in range(NCH):
                p2 = psm.tile([P, 1], f32, tag="p2", name="p2")
                nc.tensor.matmul(p2, wc2_t[co], y1,
                                 start=True, stop=True)
                nc.scalar.activation(out=s_t[co], in_=p2, func=AF.Sigmoid,
                                     bias=bc2_t[co], scale=1.0)
                nc.vector.tensor_scalar_mul(wce[co], wco_t[co], s_t[co])
            for bi in range(len(Q_EDGES) - 1):
                off, end = Q_EDGES[bi], Q_EDGES[bi + 1]
                sz = end - off
                pco = psco.tile([OC, 512], f32, tag="pco", name="pco")[:, 0:sz]
                for ci in range(NCH):
                    nc.tensor.matmul(pco, wce[ci], g_t[ci][:, off:end],
                                     start=(ci == 0), stop=(ci == NCH - 1))
                zc = pce.tile([OC, 512], f32, tag="zc", name="zc")[:, 0:sz]
                nc.scalar.activation(out=zc, in_=pco, func=AF.Identity,
                                     bias=ct['bco'], scale=1.0)
                # total = pam_out + cam_out, accumulated in place
                nc.vector.tensor_add(pam_sb[:, off:end], pam_sb[:, off:end], zc)


def _emit_upsample(nc, tc, pam_sb, y_d):
    with tc.tile_pool(name="up", bufs=1) as pu:
        su = pam_sb.rearrange("p (a b) -> p a b", b=W)  # [OC,34,64]
        a_t = pu.tile([OC, OWN_ROWS, W], f32, name="a_t", tag="a_t")
        b_t = pu.tile([OC, OWN_ROWS, W], f32, name="b_t", tag="b_t")
        nc.vector.tensor_scalar_mul(a_t.rearrange("p a b -> p (a b)"), pam_sb, 0.75)
        nc.vector.tensor_scalar_mul(b_t.rearrange("p a b -> p (a b)"), pam_sb, 0.25)
        sh = pu.tile([OC, OWN_ROWS, W, 2], f32, name="sh", tag="sh")
        nc.vector.tensor_copy(out=sh[:, :, 0, 0], in_=su[:, :, 0])
        nc.vector.tensor_add(sh[:, :, 1:W, 0], b_t[:, :, 0:W - 1], a_t[:, :, 1:W])
        nc.vector.tensor_add(sh[:, :, 0:W - 1, 1], a_t[:, :, 0:W - 1], b_t[:, :, 1:W])
        nc.vector.tensor_copy(out=sh[:, :, W - 1, 1], in_=su[:, :, W - 1])
        au = pu.tile([OC, OWN_ROWS, 2 * W], f32, name="au", tag="au")
        bu = pu.tile([OC, OWN_ROWS, 2 * W], f32, name="bu", tag="bu")
        shf = sh.rearrange("p a b c -> p a (b c)")
        nc.vector.tensor_scalar_mul(au.rearrange("p a b -> p (a b)"),
                                    shf.rearrange("p a b -> p (a b)"), 0.75)
        nc.vector.tensor_scalar_mul(bu.rearrange("p a b -> p (a b)"),
                                    shf.rearrange("p a b -> p (a b)"), 0.25)
        out_t = pu.tile([OC, H // 2, 2, 2 * W], f32, name="out_t", tag="out_t")
        nc.vector.tensor_copy(out=out_t[:, 0, 0, :], in_=shf[:, 0, :])
        nc.vector.tensor_add(out_t[:, 1:H // 2, 0, :], bu[:, 0:H // 2 - 1, :],
                             au[:, 1:H // 2, :])
        nc.vector.tensor_add(out_t[:, 0:H // 2, 1, :], au[:, 0:H // 2, :],
                             bu[:, 1:H // 2 + 1, :])
        nc.sync.dma_start(out=y_d, in_=out_t.rearrange("p a b c -> p (a b) c"))


def _build():
    nc = bacc.Bacc("TRN2", target_bir_lowering=False, debug=False,
                   enable_asserts=True, num_devices=8)

    def din(name, shape, dtp=f32):
        return nc.dram_tensor(name, shape, dtp, kind="ExternalInput").ap()

    d = {
        'xh': din("xh", [NCH, P, H, W], bf16),
        'xl': din("xl", [NCH, P, H, W], bf16),
        'xs': din("xs", [NCH, P, H, W], fp16),
        'wph': din("wph", [NCH, NT, P, P], bf16),
        'wpl': din("wpl", [NCH, NT, P, P], bf16),
        'wcm': din("wcm", [NCH, NT, P, P], fp16),
        'sp': din("sp", [NCH, P, 1]), 'bp': din("bp", [NCH, P, 1]),
        'sc': din("sc", [NCH, P, 1]), 'bc': din("bc", [NCH, P, 1]),
        'wqh': din("wqh", [NCH, P, CR], bf16), 'wql': din("wql", [NCH, P, CR], bf16),
        'wkh': din("wkh", [NCH, P, CR], bf16), 'wkl': din("wkl", [NCH, P, CR], bf16),
        'bq': din("bq", [CR, 1]), 'bk': din("bk", [CR, 1]),
        'wvh': din("wvh", [NCH, P, C], bf16), 'wvl': din("wvl", [NCH, P, C], bf16),
        'bv': din("bv", [1, C]),
        'alpha': din("alpha", [1, 1]),
        'wpoh': din("wpoh", [NCH, P, OC], bf16),
        'wpol': din("wpol", [NCH, P, OC], bf16),
        'bpo': din("bpo", [OC, 1]),
        'wc1': din("wc1", [NCH, P, CR]), 'bc1': din("bc1", [CR, 1]),
        'wc2': din("wc2", [NCH, CR, P]), 'bc2': din("bc2", [NCH, P, 1]),
        'wco': din("wco", [NCH, P, OC]), 'bco': din("bco", [OC, 1]),
        'ident': din("ident", [P, P], fp16),
    }
    y_d = nc.dram_tensor("y", [OC, H, 2 * W], f32, kind="ExternalOutput").ap()

    with tile.TileContext(nc) as tc:
        with tc.tile_pool(name="consts", bufs=2) as pc, \
             tc.tile_pool(name="fdram", bufs=1, space="DRAM") as pfd:
            ct = {}
            ct['ident'] = pc.tile([P, P], fp16, name="ident", tag="ident")
            nc.sync.dma_start(out=ct['ident'], in_=d['ident'])
            ct['alpha'] = pc.tile([P, 1], f32, name="alpha_t", tag="alpha_t")
            nc.sync.dma_start(out=ct['alpha'], in_=d['alpha'].to_broadcast([P, 1]))
            for nm, rows in (('bq', CR), ('bk', CR), ('bpo', OC), ('bco', OC),
                             ('bc1', CR)):
                ct[nm] = pc.tile([rows, 1], f32, name=f"{nm}_t", tag=f"{nm}_t")
                nc.sync.dma_start(out=ct[nm], in_=d[nm])
            for nm in ('sp', 'bp', 'sc', 'bc'):
                ct[nm] = [pc.tile([P, 1], f32, name=f"{nm}{i}_t", tag=f"{nm}{i}_t") for i in range(NCH)]
                for i in range(NCH):
                    nc.sync.dma_start(out=ct[nm][i], in_=d[nm][i])
            for nm in ('wpoh', 'wpol'):
                ct[nm] = [pc.tile([P, OC], bf16, name=f"{nm}{i}_t", tag=f"{nm}{i}_t")
                          for i in range(NCH)]
                for i in range(NCH):
                    nc.sync.dma_start(out=ct[nm][i], in_=d[nm][i])

            fsc = pfd.tile([NCH, 2, P, OWN], bf16, name="fsc", tag="fsc")
            qsc = pfd.tile([CR, OWN], f32, name="qsc", tag="qsc")
            ksc = pfd.tile([CR, HW], f32, name="ksc", tag="ksc")
            vsc = pfd.tile([P, NJC, C], fp16, name="vsc", tag="vsc")

            with tc.tile_pool(name="f_store", bufs=1) as p_f:
                fha = [p_f.tile([P, OWN], bf16, name=f"fha{i}", tag=f"fha{i}") for i in range(NCH)]
                fla = [p_f.tile([P, OWN], bf16, name=f"fla{i}", tag=f"fla{i}") for i in range(NCH)]
                fhb = [p_f.tile([P, HW - OWN], bf16, name=f"fhb{i}", tag=f"fhb{i}")
                       for i in range(NCH)]
                flb = [p_f.tile([P, HW - OWN], bf16, name=f"flb{i}", tag=f"flb{i}")
                       for i in range(NCH)]

                def f_store(co, off, src):
                    """split src ([128, n] f32 AP) into the bf16 hi/lo store."""
                    n = src.shape[-1]
                    pieces = []
                    if off < OWN:
                        k = min(OWN - off, n)
                        pieces.append((fha[co], fla[co], off, 0, k))
                    if off + n > OWN:
                        s_loc = max(OWN, off)
                        pieces.append((fhb[co], flb[co], s_loc - OWN, s_loc - off,
                                       off + n - s_loc))
                    for hi_t, lo_t, d0, s0, ln in pieces:
                        nc.vector.tensor_copy(out=hi_t[:, d0:d0 + ln],
                                              in_=src[:, s0:s0 + ln])
                        nc.vector.tensor_sub(lo_t[:, d0:d0 + ln],
                                             src[:, s0:s0 + ln],
                                             hi_t[:, d0:d0 + ln])

                def f_read(hi, co, s, e):
                    """AP for f[co][:, s:e]; must not cross the OWN boundary."""
                    if e <= OWN:
                        return (fha[co] if hi else fla[co])[:, s:e]
                    assert s >= OWN
                    return (fhb[co] if hi else flb[co])[:, s - OWN:e - OWN]

                _emit_phase1(nc, tc, d, ct, f_store)
                _emit_phase2(nc, tc, d, ct, f_read, qsc, ksc, vsc)
                # spill own-range features for residual streaming
                for co in range(NCH):
                    nc.sync.dma_start(out=fsc[co, 0], in_=fha[co])
                    nc.sync.dma_start(out=fsc[co, 1], in_=fla[co])

            with tc.tile_pool(name="pam_out", bufs=1) as p_pam:
                pam_sb = p_pam.tile([OC, OWN], f32, name="pam_sb", tag="pam_sb")
                _emit_attention(nc, tc, ct, pam_sb, fsc, qsc, ksc, vsc)
                _emit_cam(nc, tc, d, ct, pam_sb)
                _emit_upsample(nc, tc, pam_sb, y_d)
    nc.compile()
    return nc


_NC_CACHE = None


def _get_nc():
    global _NC_CACHE
    if _NC_CACHE is None:
        _NC_CACHE = _build()
    return _NC_CACHE


def _hi_lo(a):
    hi = np.asarray(a, np.float32).astype(BF)
    lo = (np.asarray(a, np.float32) - hi.astype(np.float32)).astype(BF)
    return hi, lo


_TAP_CI = np.array([t[0] for t in TAPS])
_TAP_DY = np.array([t[1] + 1 for t in TAPS])
_TAP_DX = np.array([t[2] + 1 for t in TAPS])


def _pack_conv(wfull):
    """[C, C, 3, 3] -> [NCH(co), NT, P(ci_local), P(co_local)] lhsT tiles."""
    wr = np.asarray(wfull, np.float32).reshape(NCH, P, NCH, P, 3, 3)
    wt = wr.transpose(0, 2, 4, 5, 3, 1)  # [co, ci, dy, dx, ci_l, co_l]
    return np.ascontiguousarray(wt[:, _TAP_CI, _TAP_DY, _TAP_DX])


def _packT(w, free):
    """w [free, C] -> [NCH, P, free] lhsT chunks."""
    return np.ascontiguousarray(np.asarray(w, np.float32).T.reshape(NCH, P, free))


def _prep_shared(inputs, flip):
    wp = np.asarray(inputs['W_pam_in'], np.float32)
    wc = np.asarray(inputs['W_cam_in'], np.float32)
    if flip:
        wp = wp[:, :, ::-1, :]
        wc = wc[:, :, ::-1, :]
    wph, wpl = _hi_lo(_pack_conv(wp))
    wcm = _pack_conv(wc).astype(np.float16)

    def bnfold(g, b, m, v):
        s = (np.asarray(g, np.float32)
             / np.sqrt(np.asarray(v, np.float32) + EPS)).astype(np.float32)
        bb = (np.asarray(b, np.float32)
              - np.asarray(m, np.float32) * s).astype(np.float32)
        return s.reshape(NCH, P, 1), bb.reshape(NCH, P, 1)

    sp, bp = bnfold(inputs['pam_gamma'], inputs['pam_beta'],
                    inputs['pam_mean'], inputs['pam_var'])
    sc, bc = bnfold(inputs['cam_gamma'], inputs['cam_beta'],
                    inputs['cam_mean'], inputs['cam_var'])
    wqh, wql = _hi_lo(_packT(inputs['Wq'], CR))
    wkh, wkl = _hi_lo(_packT(inputs['Wk'], CR))
    wvh, wvl = _hi_lo(_packT(inputs['Wv'], C))
    wpoh, wpol = _hi_lo(_packT(inputs['W_pam_out'], OC))
    # Wc2 [C, CR] -> lhsT chunks [NCH, CR, P]
    wc2 = np.ascontiguousarray(
        np.asarray(inputs['Wc2'], np.float32).reshape(NCH, P, CR).transpose(0, 2, 1))
    return {
        'wph': wph, 'wpl': wpl, 'wcm': wcm,
        'sp': sp, 'bp': bp, 'sc': sc, 'bc': bc,
        'wqh': wqh, 'wql': wql, 'wkh': wkh, 'wkl': wkl,
        'bq': np.asarray(inputs['bq'], np.float32).reshape(CR, 1),
        'bk': np.asarray(inputs['bk'], np.float32).reshape(CR, 1),
        'wvh': wvh, 'wvl': wvl,
        'bv': np.asarray(inputs['bv'], np.float32).reshape(1, C),
        'alpha': np.asarray(inputs['alpha'], np.float32).reshape(1, 1),
        'wpoh': wpoh, 'wpol': wpol,
        'bpo': np.asarray(inputs['b_pam_out'], np.float32).reshape(OC, 1),
        'wc1': _packT(np.asarray(inputs['Wc1'], np.float32) / HW, CR),
        'bc1': np.asarray(inputs['bc1'], np.float32).reshape(CR, 1),
        'wc2': wc2,
        'bc2': np.asarray(inputs['bc2'], np.float32).reshape(NCH, P, 1),
        'wco': _packT(inputs['W_cam_out'], OC),
        'ident': np.eye(P, dtype=np.float16),
        'bco': np.asarray(inputs['b_cam_out'], np.float32).reshape(OC, 1),
    }


def _make_in_maps(inputs):
    x = np.asarray(inputs['x'], np.float32)  # [4, 512, 64, 64]
    shared = {f: _prep_shared(inputs, f) for f in (False, True)}
    in_maps = []
    for c in range(8):
        s, flip = c // 2, c % 2
        xs = x[s]
        if flip:
            xs = xs[:, ::-1, :]
        xs = np.ascontiguousarray(xs.reshape(NCH, P, H, W))
        xhh, xll = _hi_lo(xs)
        m = dict(shared[bool(flip)])
        m['xs'] = xs.astype(np.float16)
        m['xh'] = xhh
        m['xl'] = xll
        in_maps.append(m)
    return in_maps


def kernel(**inputs):
    nc = _get_nc()
    in_maps = _make_in_maps(inputs)
    res = run_bass_kernel_spmd(nc, in_maps, list(range(8)))
    out = np.empty((4, OC, 2 * H, 2 * W), np.float32)
    for c in range(8):
        s, flip = c // 2, c % 2
        o = res.results[c]['y']  # [64, 64, 128]
        if flip:
            out[s, :, H:2 * H, :] = o[:, ::-1, :]
        else:
            out[s, :, 0:H, :] = o
    return out



# revision 19
# speedup vs baseline: 2.1539x; 2.1539x over previous
"""DAHead (dual-attention head) Trainium2 kernel, v2.

8-core SPMD with pair collectives: core c handles sample c//2, spatial
half c%2. Odd cores see the sample vertically flipped (conv weights
flipped along dy) so every core runs the same program; the host
un-flips the output half.

v2 vs v1: the conv3x3 / q/k/v work is split across the sample's core
PAIR: each core computes f and g only on its own 34 rows, k/v only on
its own 32 rows, and the pair AllGathers k (f32, 0.5MB) and vT (fp16,
2MB) plus AllReduces the CAM channel-mean, overlapped with the CAM
conv. Attention j-order becomes [pair-rank0 rows, pair-rank1 rows],
a permutation of the 4096 tokens - softmax+apply are
permutation-invariant over j.

Precision: the softmax is argmax-like (logits up to ~375), so the
q/k path runs in f32r (full-rate PE mode, ~1.6e-4 matmul error vs
fp16's 2.9e-4): PAM conv f32r -> f kept f32 in SBUF -> q/k f32r ->
logits f32r. The linear paths (v, attention apply, residual,
out-projs, whole CAM branch) are fp16/f32r single-pass.

Per-core program:
  1. conv3x3+BN+lrelu (PAM) f32r, rows 0..33 -> f f32 in SBUF.
  2. k (rows 0..31) f32, vT (rows 0..31) fp16, q (rows 0..33) f32;
     k/vT -> DRAM -> pair AllGather.
  3. conv3x3+BN+lrelu (CAM) f32r rows 0..33 -> g fp16 in SBUF;
     partial channel sums over rows 0..31 -> pair AllReduce.
  4. Attention over own 17 i-chunks: f32r logits, row softmax (max on
     gpsimd, exp+scale on ACT), PE transpose of prob chunks, fp16
     apply, residual fused to fp16, 1x1 out-proj -> pam_sb f32.
  5. CAM MLP (sigmoid folded into the 1x1 weights), out-proj
     accumulated into pam_sb.
  6. bilinear x2 upsample (DVE) of rows 0..32, DMA out.
"""
import sys

if '/opt/trn_rl_repo' not in sys.path:
    sys.path.insert(0, '/opt/trn_rl_repo')

import numpy as np

import concourse.bass as bass
import concourse.mybir as mybir
import concourse.tile as tile
from concourse import bacc
from concourse.bass_utils import run_bass_kernel_spmd

dt = mybir.dt
f32 = dt.float32
f32r = dt.float32r
fp16 = dt.float16
AF = mybir.ActivationFunctionType
OP = mybir.AluOpType

C = 512          # channels
P = 128          # partition size
NCH = C // P     # channel chunks (4)
H = W = 64
HW = H * W       # 4096
CR = 64          # q/k channels
OC = 64          # output channels
OWN_ROWS = 34    # rows computed per core (0..33); rows 32+ feed upsample only
OWN = OWN_ROWS * W    # 2176 = 17*128
NIC = OWN // P        # 17 attention i-chunks
KV_ROWS = 32          # rows contributed to the gathered k/v (0..31)
KOWN = KV_ROWS * W    # 2048
KJC = KOWN // P       # 16 own j-chunks
NJC = HW // P         # 32 gathered j-chunks
XR = OWN_ROWS + 1     # x rows needed (0..34: +1 halo row below)
XW = W + 2            # x cols incl zero-pad columns (f32r needs even APs)
EPS = 1e-5
RG = [[0, 1], [2, 3], [4, 5], [6, 7]]   # sample pairs

# conv h-blocks (start_row, n_rows); 7-row blocks keep the moving dim
# >= 256 even with dx-edge clamps (f32r full-rate needs >= 256)
HBLK = [(0, 7), (7, 7), (14, 7), (21, 7), (28, 6)]

# tap order: full-coverage center tap first (needed for PSUM start flag)
_ALL = [(ci, dy, dx) for ci in range(NCH) for dy in (-1, 0, 1) for dx in (-1, 0, 1)]
TAPS = [(0, 0, 0)] + [t for t in _ALL if t != (0, 0, 0)]
NT = len(TAPS)   # 36

Q_EDGES = [0, 512, 1024, 1536, 2048, OWN]
K_EDGES = [0, 512, 1024, 1536, 2048]

# attention i-blocks (first_chunk, n_chunks)
IBLK = [(0, 2), (2, 2), (4, 2), (6, 2), (8, 2), (10, 2), (12, 2), (14, 2), (16, 1)]


def _conv_tap_aps(psum_t, x_t, r0b, nr, dy, dx):
    """APs for one conv tap on block rows [r0b, r0b+nr). x_t: [128,XR,XW]
    with zero-padded columns 0 and 65 (f32r needs even-width APs)."""
    r0 = 1 if (r0b == 0 and dy == -1) else 0
    out_ap = psum_t[:, r0:nr, :]
    in_ap = x_t[:, r0b + r0 + dy: r0b + nr + dy, 1 + dx: 1 + dx + W]
    return out_ap, in_ap


def _emit_conv(nc, tc, w_d, s_ct, b_ct, x_t, dst, odt, pool_name):
    """conv3x3+BN+lrelu f32r over rows 0..33 -> dst[co] [P, OWN] (odt)."""
    with tc.tile_pool(name=pool_name, bufs=2) as pw, \
         tc.tile_pool(name=pool_name + "_ev", bufs=3) as pe, \
         tc.tile_pool(name=pool_name + "_ps", bufs=1, space="PSUM") as psc:
        for co in range(NCH):
            wt = pw.tile([P, NT, P], f32r, tag="w", name="w")
            nc.sync.dma_start(out=wt, in_=w_d[co].rearrange("t p f -> p t f"))
            for b, (r0b, nr) in enumerate(HBLK):
                pst = psc.tile([P, nr, W], f32, tag="cv", name="cv")
                for t, (ci, dy, dx) in enumerate(TAPS):
                    o_ap, i_ap = _conv_tap_aps(pst, x_t[ci], r0b, nr, dy, dx)
                    nc.tensor.matmul(o_ap, wt[:, t, :], i_ap,
                                     start=(t == 0), stop=(t == NT - 1))
                z = pe.tile([P, 7 * W], odt, tag="z", name="z")[:, 0:nr * W]
                nc.scalar.activation(
                    out=z, in_=pst.rearrange("p a b -> p (a b)"),
                    func=AF.Identity, bias=b_ct[co], scale=s_ct[co])
                nc.vector.scalar_tensor_tensor(
                    out=dst[co][:, r0b * W:(r0b + nr) * W], in0=z, scalar=0.2,
                    in1=z, op0=OP.mult, op1=OP.max)


def _emit_proj(nc, tc, d, ct, f_t, f16_t, q_t, k_own, ksc, vsc):
    """k/v own-row projections -> DRAM (gathered later), q -> SBUF."""
    with tc.tile_pool(name="proj_ev", bufs=2) as pve, \
         tc.tile_pool(name="proj_ps", bufs=2, space="PSUM") as psq:
        # k first so its AllGather starts earliest
        for bi in range(len(K_EDGES) - 1):
            off, end = K_EDGES[bi], K_EDGES[bi + 1]
            pq = psq.tile([CR, 512], f32, tag="pq", name="pq")
            for ci in range(NCH):
                nc.tensor.matmul(pq, ct['wk'][ci], f_t[ci][:, off:end],
                                 start=(ci == 0), stop=(ci == NCH - 1))
            nc.scalar.activation(out=k_own[:, off:end], in_=pq,
                                 func=AF.Identity, bias=ct['bk'], scale=1.0)
        nc.sync.dma_start(out=ksc, in_=k_own)

        for jc in range(KJC):
            pv = psq.tile([P, C], f32, tag="pv", name="pv")
            s, e = jc * P, (jc + 1) * P
            for ci in range(NCH):
                nc.tensor.matmul(pv, f16_t[ci][:, s:e], ct['wv'][ci],
                                 start=(ci == 0), stop=(ci == NCH - 1))
            vtmp = pve.tile([P, C], fp16, tag="vtmp", name="vtmp")
            nc.vector.tensor_add(vtmp, pv, ct['bv'])
            nc.sync.dma_start(out=vsc[:, jc, :], in_=vtmp)

        for bi in range(len(Q_EDGES) - 1):
            off, end = Q_EDGES[bi], Q_EDGES[bi + 1]
            pq = psq.tile([CR, 512], f32, tag="pq", name="pq")[:, 0:end - off]
            for ci in range(NCH):
                nc.tensor.matmul(pq, ct['wq'][ci], f_t[ci][:, off:end],
                                 start=(ci == 0), stop=(ci == NCH - 1))
            nc.scalar.activation(out=q_t[:, off:end], in_=pq,
                                 func=AF.Identity, bias=ct['bq'], scale=1.0)


def _emit_attention(nc, tc, ct, f_t, q_t, pam_sb, kg, vg):
    with tc.tile_pool(name="kv2", bufs=1) as pv2, \
         tc.tile_pool(name="ls", bufs=1) as pls, \
         tc.tile_pool(name="e16p", bufs=2) as pep, \
         tc.tile_pool(name="et", bufs=2) as pet, \
         tc.tile_pool(name="att_tmp", bufs=2) as pat, \
         tc.tile_pool(name="res_t", bufs=2) as prs, \
         tc.tile_pool(name="ps_l", bufs=2, space="PSUM") as psl, \
         tc.tile_pool(name="ps_t", bufs=2, space="PSUM") as pstp, \
         tc.tile_pool(name="ps_a", bufs=2, space="PSUM") as psa, \
         tc.tile_pool(name="ps_p", bufs=2, space="PSUM") as psp:
        k_t = pv2.tile([CR, HW], f32r, name="k2_t", tag="k2_t")
        vt_t = pv2.tile([P, NJC, C], fp16, name="vt2_t", tag="vt2_t")
        nc.sync.dma_start(out=k_t[:, 0:KOWN], in_=kg[0])
        nc.sync.dma_start(out=k_t[:, KOWN:HW], in_=kg[1])
        nc.sync.dma_start(out=vt_t[:, 0:KJC, :], in_=vg[0])
        nc.sync.dma_start(out=vt_t[:, KJC:NJC, :], in_=vg[1])
        for ib, (ic0, nch) in enumerate(IBLK):
            isz = P * nch
            ioff = ic0 * P
            et_t = pet.tile([P, NJC, 2 * P], fp16, tag="et", name="et")
            for ph in range(nch):
                ic = ic0 + ph
                ls = pls.tile([P, HW], f32, tag="ls", name="ls")
                for jb in range(HW // 512):
                    pl = psl.tile([P, 512], f32, tag="pl", name="pl")
                    nc.tensor.matmul(
                        pl, q_t[:, ic * P:(ic + 1) * P],
                        k_t[:, jb * 512:(jb + 1) * 512], start=True, stop=True)
                    nc.scalar.activation(
                        out=ls[:, jb * 512:(jb + 1) * 512], in_=pl,
                        func=AF.Identity, bias=0.0, scale=1.0)
                nmax = pat.tile([P, 1], f32, tag="nmax", name="nmax")
                nc.vector.tensor_reduce(out=nmax, in_=ls, axis=mybir.AxisListType.X,
                                        op=OP.max, negate=True)
                rsum = pat.tile([P, 1], f32, tag="rsum", name="rsum")
                nc.scalar.activation(out=ls, in_=ls, func=AF.Exp,
                                     bias=nmax, scale=1.0, accum_out=rsum)
                rrec = pat.tile([P, 1], f32, tag="rrec", name="rrec")
                nc.vector.reciprocal(out=rrec, in_=rsum)
                e16 = pep.tile([P, HW], fp16, tag="e16", name="e16")
                nc.scalar.activation(out=e16, in_=ls, func=AF.Identity,
                                     bias=0.0, scale=rrec)
                for jc in range(NJC):
                    pt = pstp.tile([P, P], fp16, tag="pt", name="pt")
                    nc.tensor.transpose(
                        pt, e16[:, jc * P:(jc + 1) * P], ct['ident'])
                    if jc % 2 == 0:
                        nc.vector.tensor_copy(
                            out=et_t[:, jc, ph * P:(ph + 1) * P], in_=pt)
                    else:
                        nc.scalar.activation(
                            out=et_t[:, jc, ph * P:(ph + 1) * P], in_=pt,
                            func=AF.Identity, bias=0.0, scale=1.0)
            r16 = []
            for co in range(NCH):
                pa = psa.tile([P, 2 * P], f32, tag="pa", name="pa")[:, 0:isz]
                for jc in range(NJC):
                    nc.tensor.matmul(
                        pa, vt_t[:, jc, co * P:(co + 1) * P],
                        et_t[:, jc, 0:isz],
                        start=(jc == 0), stop=(jc == NJC - 1))
                rt = prs.tile([P, 2 * P], fp16, tag=f"r{co}", name=f"r{co}")[:, 0:isz]
                nc.vector.scalar_tensor_tensor(
                    out=rt, in0=pa, scalar=ct['alpha'],
                    in1=f_t[co][:, ioff:ioff + isz], op0=OP.mult, op1=OP.add)
                r16.append(rt)
            pp = psp.tile([OC, 2 * P], f32, tag="pp", name="pp")[:, 0:isz]
            for ci in range(NCH):
                nc.tensor.matmul(pp, ct['wpo'][ci], r16[ci],
                                 start=(ci == 0), stop=(ci == NCH - 1))
            nc.scalar.activation(out=pam_sb[:, ioff:ioff + isz], in_=pp,
                                 func=AF.Identity, bias=ct['bpo'], scale=1.0)


def _emit_cam_tail(nc, tc, d, ct, g_t, pam_sb, msg):
    """channel-attention MLP (mean already AllReduced into msg) + 1x1 out."""
    with tc.tile_pool(name="mlp", bufs=1) as pm, \
         tc.tile_pool(name="cam_ev", bufs=2) as pce, \
         tc.tile_pool(name="ps_mlp", bufs=2, space="PSUM") as psm, \
         tc.tile_pool(name="ps_co", bufs=2, space="PSUM") as psco:
        msum = [pm.tile([P, 1], f32, name=f"ms{i}", tag=f"ms{i}") for i in range(NCH)]
        for i in range(NCH):
            nc.sync.dma_start(out=msum[i], in_=msg[i])
        wc1_t = [pm.tile([P, CR], f32, name=f"w1{i}", tag=f"w1{i}") for i in range(NCH)]
        wc2_t = [pm.tile([CR, P], f32, name=f"w2{i}", tag=f"w2{i}") for i in range(NCH)]
        wco_t = [pm.tile([P, OC], f32, name=f"wo{i}", tag=f"wo{i}") for i in range(NCH)]
        bc2_t = [pm.tile([P, 1], f32, name=f"b2{i}", tag=f"b2{i}") for i in range(NCH)]
        for i in range(NCH):
            nc.sync.dma_start(out=wc1_t[i], in_=d['wc1'][i])
            nc.sync.dma_start(out=wc2_t[i], in_=d['wc2'][i])
            nc.sync.dma_start(out=wco_t[i], in_=d['wco'][i])
            nc.sync.dma_start(out=bc2_t[i], in_=d['bc2'][i])
        p1 = psm.tile([CR, 1], f32, tag="p1", name="p1")
        for ci in range(NCH):
            nc.tensor.matmul(p1, wc1_t[ci], msum[ci],
                             start=(ci == 0), stop=(ci == NCH - 1))
        t1 = pm.tile([CR, 1], f32, name="t1", tag="t1")
        nc.scalar.activation(out=t1, in_=p1, func=AF.Identity,
                             bias=ct['bc1'], scale=1.0)
        y1 = pm.tile([CR, 1], f32, name="y1", tag="y1")
        nc.vector.scalar_tensor_tensor(out=y1, in0=t1, scalar=0.2, in1=t1,
                                       op0=OP.mult, op1=OP.max)
        wce = [pm.tile([P, OC], fp16, name=f"we{i}", tag=f"we{i}") for i in range(NCH)]
        for co in range(NCH):
            p2 = psm.tile([P, 1], f32, tag="p2", name="p2")
            nc.tensor.matmul(p2, wc2_t[co], y1, start=True, stop=True)
            s_t = pm.tile([P, 1], f32, name=f"s{co}", tag=f"s{co}")
            nc.scalar.activation(out=s_t, in_=p2, func=AF.Sigmoid,
                                 bias=bc2_t[co], scale=1.0)
            nc.vector.tensor_scalar_mul(wce[co], wco_t[co], s_t)
        for bi in range(len(Q_EDGES) - 1):
            off, end = Q_EDGES[bi], Q_EDGES[bi + 1]
            sz = end - off
            pco = psco.tile([OC, 512], f32, tag="pco", name="pco")[:, 0:sz]
            for ci in range(NCH):
                nc.tensor.matmul(pco, wce[ci], g_t[ci][:, off:end],
                                 start=(ci == 0), stop=(ci == NCH - 1))
            zc = pce.tile([OC, 512], f32, tag="zc", name="zc")[:, 0:sz]
            nc.scalar.activation(out=zc, in_=pco, func=AF.Identity,
                                 bias=ct['bco'], scale=1.0)
            nc.vector.tensor_add(pam_sb[:, off:end], pam_sb[:, off:end], zc)


def _emit_upsample(nc, tc, pam_sb, y_d):
    with tc.tile_pool(name="up", bufs=1) as pu:
        su = pam_sb.rearrange("p (a b) -> p a b", b=W)  # [OC,34,64]
        a_t = pu.tile([OC, OWN_ROWS, W], f32, name="a_t", tag="a_t")
        b_t = pu.tile([OC, OWN_ROWS, W], f32, name="b_t", tag="b_t")
        nc.vector.tensor_scalar_mul(a_t.rearrange("p a b -> p (a b)"), pam_sb, 0.75)
        nc.vector.tensor_scalar_mul(b_t.rearrange("p a b -> p (a b)"), pam_sb, 0.25)
        sh = pu.tile([OC, OWN_ROWS, W, 2], f32, name="sh", tag="sh")
        nc.vector.tensor_copy(out=sh[:, :, 0, 0], in_=su[:, :, 0])
        nc.vector.tensor_add(sh[:, :, 1:W, 0], b_t[:, :, 0:W - 1], a_t[:, :, 1:W])
        nc.vector.tensor_add(sh[:, :, 0:W - 1, 1], a_t[:, :, 0:W - 1], b_t[:, :, 1:W])
        nc.vector.tensor_copy(out=sh[:, :, W - 1, 1], in_=su[:, :, W - 1])
        au = pu.tile([OC, OWN_ROWS, 2 * W], f32, name="au", tag="au")
        bu = pu.tile([OC, OWN_ROWS, 2 * W], f32, name="bu", tag="bu")
        shf = sh.rearrange("p a b c -> p a (b c)")
        nc.vector.tensor_scalar_mul(au.rearrange("p a b -> p (a b)"),
                                    shf.rearrange("p a b -> p (a b)"), 0.75)
        nc.vector.tensor_scalar_mul(bu.rearrange("p a b -> p (a b)"),
                                    shf.rearrange("p a b -> p (a b)"), 0.25)
        out_t = pu.tile([OC, H // 2, 2, 2 * W], f32, name="out_t", tag="out_t")
        nc.vector.tensor_copy(out=out_t[:, 0, 0, :], in_=shf[:, 0, :])
        nc.vector.tensor_add(out_t[:, 1:H // 2, 0, :], bu[:, 0:H // 2 - 1, :],
                             au[:, 1:H // 2, :])
        nc.vector.tensor_add(out_t[:, 0:H // 2, 1, :], au[:, 0:H // 2, :],
                             bu[:, 1:H // 2 + 1, :])
        nc.sync.dma_start(out=y_d, in_=out_t.rearrange("p a b c -> p (a b) c"))


def _build():
    nc = bacc.Bacc("TRN2", target_bir_lowering=False, debug=False,
                   enable_asserts=True, num_devices=8)

    def din(name, shape, dtp=f32):
        return nc.dram_tensor(name, shape, dtp, kind="ExternalInput").ap()

    d = {
        'xs': din("xs", [NCH, P, XR, XW], f32r),
        'wpm': din("wpm", [NCH, NT, P, P], f32r),
        'wcm': din("wcm", [NCH, NT, P, P], f32r),
        'sp': din("sp", [NCH, P, 1]), 'bp': din("bp", [NCH, P, 1]),
        'sc': din("sc", [NCH, P, 1]), 'bc': din("bc", [NCH, P, 1]),
        'wq': din("wq", [NCH, P, CR], f32r), 'wk': din("wk", [NCH, P, CR], f32r),
        'bq': din("bq", [CR, 1]), 'bk': din("bk", [CR, 1]),
        'wv': din("wv", [NCH, P, C], fp16), 'bv': din("bv", [1, C]),
        'alpha': din("alpha", [1, 1]),
        'wpo': din("wpo", [NCH, P, OC], fp16), 'bpo': din("bpo", [OC, 1]),
        'wc1': din("wc1", [NCH, P, CR]), 'bc1': din("bc1", [CR, 1]),
        'wc2': din("wc2", [NCH, CR, P]), 'bc2': din("bc2", [NCH, P, 1]),
        'wco': din("wco", [NCH, P, OC]), 'bco': din("bco", [OC, 1]),
        'ident': din("ident", [P, P], fp16),
    }
    y_d = nc.dram_tensor("y", [OC, H, 2 * W], f32, kind="ExternalOutput").ap()

    with tile.TileContext(nc) as tc:
        with tc.tile_pool(name="consts", bufs=1) as pc, \
             tc.tile_pool(name="fdram", bufs=1, space="DRAM") as pfd:
            ct = {}
            ct['ident'] = pc.tile([P, P], fp16, name="ident", tag="ident")
            nc.sync.dma_start(out=ct['ident'], in_=d['ident'])
            ct['alpha'] = pc.tile([P, 1], f32, name="alpha_t", tag="alpha_t")
            nc.sync.dma_start(out=ct['alpha'], in_=d['alpha'].to_broadcast([P, 1]))
            ct['bv'] = pc.tile([P, C], f32, name="bv_t", tag="bv_t")
            nc.sync.dma_start(out=ct['bv'], in_=d['bv'].to_broadcast([P, C]))
            for nm, rows in (('bq', CR), ('bk', CR), ('bpo', OC), ('bco', OC),
                             ('bc1', CR)):
                ct[nm] = pc.tile([rows, 1], f32, name=f"bias_{nm}", tag=f"bias_{nm}")
                nc.sync.dma_start(out=ct[nm], in_=d[nm])
            for nm in ('sp', 'bp', 'sc', 'bc'):
                ct[nm] = [pc.tile([P, 1], f32, name=f"{nm}_{i}_t", tag=f"{nm}_{i}_t")
                          for i in range(NCH)]
                for i in range(NCH):
                    nc.sync.dma_start(out=ct[nm][i], in_=d[nm][i])
            for nm, fr, dtp in (('wq', CR, f32r), ('wk', CR, f32r),
                                ('wv', C, fp16), ('wpo', OC, fp16)):
                ct[nm] = [pc.tile([P, fr], dtp, name=f"{nm}{i}_t", tag=f"{nm}{i}_t")
                          for i in range(NCH)]
                for i in range(NCH):
                    nc.sync.dma_start(out=ct[nm][i], in_=d[nm][i])

            ksc = pfd.tile([CR, KOWN], f32r, name="ksc", tag="ksc")
            kg = pfd.tile([2, CR, KOWN], f32r, name="kg", tag="kg")
            vsc = pfd.tile([P, KJC, C], fp16, name="vsc", tag="vsc")
            vg = pfd.tile([2, P, KJC, C], fp16, name="vg", tag="vg")
            msc = pfd.tile([NCH, P, 1], f32, name="msc", tag="msc")
            msg = pfd.tile([NCH, P, 1], f32, name="msg", tag="msg")

            with tc.tile_pool(name="f16_store", bufs=1) as p_f16, \
                 tc.tile_pool(name="g_store", bufs=1) as p_g, \
                 tc.tile_pool(name="qk_sb", bufs=1) as p_qk, \
                 tc.tile_pool(name="pam_out", bufs=1) as p_pam:
                f16_t = [p_f16.tile([P, OWN], fp16, name=f"h{i}", tag=f"h{i}")
                         for i in range(NCH)]
                g_t = [p_g.tile([P, OWN], fp16, name=f"g{i}", tag=f"g{i}")
                       for i in range(NCH)]
                q_t = p_qk.tile([CR, OWN], f32r, name="q_t", tag="q_t")
                pam_sb = p_pam.tile([OC, OWN], f32, name="pam_sb", tag="pam_sb")

                with tc.tile_pool(name="xs_sb", bufs=1) as px, \
                     tc.tile_pool(name="f_store", bufs=1) as p_f:
                    x_t = [px.tile([P, XR, XW], f32r, name=f"x{i}", tag=f"x{i}")
                           for i in range(NCH)]
                    f_t = [p_f.tile([P, OWN], f32r, name=f"f{i}", tag=f"f{i}")
                           for i in range(NCH)]
                    k_own = p_f.tile([CR, KOWN], f32r, name="k_own", tag="k_own")
                    for i in range(NCH):
                        nc.sync.dma_start(out=x_t[i], in_=d['xs'][i])
                    _emit_conv(nc, tc, d['wpm'], ct['sp'], ct['bp'], x_t, f_t,
                               f32, "pamconv")
                    for i in range(NCH):
                        nc.vector.tensor_copy(out=f16_t[i], in_=f_t[i])
                    _emit_proj(nc, tc, d, ct, f_t, f16_t, q_t, k_own, ksc, vsc)
                    nc.gpsimd.collective_compute(
                        "AllGather", OP.bypass, replica_groups=RG,
                        ins=[ksc.opt()], outs=[kg.opt()])
                    nc.gpsimd.collective_compute(
                        "AllGather", OP.bypass, replica_groups=RG,
                        ins=[vsc.opt()], outs=[vg.opt()])
                    _emit_conv(nc, tc, d['wcm'], ct['sc'], ct['bc'], x_t, g_t,
                               fp16, "camconv")
                    with tc.tile_pool(name="msump", bufs=1) as pms:
                        for i in range(NCH):
                            ms = pms.tile([P, 1], f32, name=f"pm{i}", tag=f"pm{i}")
                            nc.vector.tensor_reduce(
                                out=ms, in_=g_t[i][:, 0:KOWN],
                                axis=mybir.AxisListType.X, op=OP.add)
                            nc.sync.dma_start(out=msc[i], in_=ms)
                        nc.gpsimd.collective_compute(
                            "AllReduce", OP.add, replica_groups=RG,
                            ins=[msc.opt()], outs=[msg.opt()])

                _emit_attention(nc, tc, ct, f16_t, q_t, pam_sb, kg, vg)
                _emit_cam_tail(nc, tc, d, ct, g_t, pam_sb, msg)
                _emit_upsample(nc, tc, pam_sb, y_d)
    nc.compile()
    return nc


_NC_CACHE = None


def _get_nc():
    global _NC_CACHE
    if _NC_CACHE is None:
        _NC_CACHE = _build()
    return _NC_CACHE


_TAP_CI = np.array([t[0] for t in TAPS])
_TAP_DY = np.array([t[1] + 1 for t in TAPS])
_TAP_DX = np.array([t[2] + 1 for t in TAPS])


def _pack_conv(wfull):
    """[C, C, 3, 3] -> [NCH(co), NT, P(ci_local), P(co_local)] lhsT tiles."""
    wr = np.asarray(wfull, np.float32).reshape(NCH, P, NCH, P, 3, 3)
    wt = wr.transpose(0, 2, 4, 5, 3, 1)  # [co, ci, dy, dx, ci_l, co_l]
    return np.ascontiguousarray(wt[:, _TAP_CI, _TAP_DY, _TAP_DX])


def _packT(w, free):
    """w [free, C] -> [NCH, P, free] lhsT chunks."""
    return np.ascontiguousarray(np.asarray(w, np.float32).T.reshape(NCH, P, free))


def _prep_shared(inputs, flip):
    wp = np.asarray(inputs['W_pam_in'], np.float32)
    wc = np.asarray(inputs['W_cam_in'], np.float32)
    if flip:
        wp = wp[:, :, ::-1, :]
        wc = wc[:, :, ::-1, :]

    def bnfold(g, b, m, v):
        s = (np.asarray(g, np.float32)
             / np.sqrt(np.asarray(v, np.float32) + EPS)).astype(np.float32)
        bb = (np.asarray(b, np.float32)
              - np.asarray(m, np.float32) * s).astype(np.float32)
        return s.reshape(NCH, P, 1), bb.reshape(NCH, P, 1)

    sp, bp = bnfold(inputs['pam_gamma'], inputs['pam_beta'],
                    inputs['pam_mean'], inputs['pam_var'])
    sc, bc = bnfold(inputs['cam_gamma'], inputs['cam_beta'],
                    inputs['cam_mean'], inputs['cam_var'])
    wc2 = np.ascontiguousarray(
        np.asarray(inputs['Wc2'], np.float32).reshape(NCH, P, CR).transpose(0, 2, 1))
    return {
        'wpm': _pack_conv(wp),
        'wcm': _pack_conv(wc),
        'sp': sp, 'bp': bp, 'sc': sc, 'bc': bc,
        'wq': _packT(inputs['Wq'], CR),
        'wk': _packT(inputs['Wk'], CR),
        'bq': np.asarray(inputs['bq'], np.float32).reshape(CR, 1),
        'bk': np.asarray(inputs['bk'], np.float32).reshape(CR, 1),
        'wv': _packT(inputs['Wv'], C).astype(np.float16),
        'bv': np.asarray(inputs['bv'], np.float32).reshape(1, C),
        'alpha': np.asarray(inputs['alpha'], np.float32).reshape(1, 1),
        'wpo': _packT(inputs['W_pam_out'], OC).astype(np.float16),
        'bpo': np.asarray(inputs['b_pam_out'], np.float32).reshape(OC, 1),
        'wc1': _packT(np.asarray(inputs['Wc1'], np.float32) / HW, CR),
        'bc1': np.asarray(inputs['bc1'], np.float32).reshape(CR, 1),
        'wc2': wc2,
        'bc2': np.asarray(inputs['bc2'], np.float32).reshape(NCH, P, 1),
        'wco': _packT(inputs['W_cam_out'], OC),
        'bco': np.asarray(inputs['b_cam_out'], np.float32).reshape(OC, 1),
        'ident': np.eye(P, dtype=np.float16),
    }


def _make_in_maps(inputs):
    x = np.asarray(inputs['x'], np.float32)  # [4, 512, 64, 64]
    shared = {f: _prep_shared(inputs, f) for f in (False, True)}
    in_maps = []
    for c in range(8):
        s, flip = c // 2, c % 2
        xs = x[s]
        if flip:
            xs = xs[:, ::-1, :]
        xp = np.zeros((C, XR, XW), np.float32)
        xp[:, :, 1:1 + W] = xs[:, 0:XR, :]
        m = dict(shared[bool(flip)])
        m['xs'] = np.ascontiguousarray(xp.reshape(NCH, P, XR, XW))
        in_maps.append(m)
    return in_maps


def kernel(**inputs):
    nc = _get_nc()
    in_maps = _make_in_maps(inputs)
    res = run_bass_kernel_spmd(nc, in_maps, list(range(8)))
    out = np.empty((4, OC, 2 * H, 2 * W), np.float32)
    for c in range(8):
        s, flip = c // 2, c % 2
        o = res.results[c]['y']  # [64, 64, 128]
        if flip:
            out[s, :, H:2 * H, :] = o[:, ::-1, :]
        else:
            out[s, :, 0:H, :] = o
    return out


# revision 22
# speedup vs baseline: 2.1686x; 1.0069x over previous
"""DAHead (dual-attention head) Trainium2 kernel, v2.

8-core SPMD with pair collectives: core c handles sample c//2, spatial
half c%2. Odd cores see the sample vertically flipped (conv weights
flipped along dy) so every core runs the same program; the host
un-flips the output half.

v2 vs v1: the conv3x3 / q/k/v work is split across the sample's core
PAIR: each core computes f and g only on its own 34 rows, k/v only on
its own 32 rows, and the pair AllGathers k (f32, 0.5MB) and vT (fp16,
2MB) plus AllReduces the CAM channel-mean, overlapped with the CAM
conv. Attention j-order becomes [pair-rank0 rows, pair-rank1 rows],
a permutation of the 4096 tokens - softmax+apply are
permutation-invariant over j.

Precision: the softmax is argmax-like (logits up to ~375), so the
q/k path runs in f32r (full-rate PE mode, ~1.6e-4 matmul error vs
fp16's 2.9e-4): PAM conv f32r -> f kept f32 in SBUF -> q/k f32r ->
logits f32r. The linear paths (v, attention apply, residual,
out-projs, whole CAM branch) are fp16/f32r single-pass.

Per-core program:
  1. conv3x3+BN+lrelu (PAM) f32r, rows 0..33 -> f f32 in SBUF.
  2. k (rows 0..31) f32, vT (rows 0..31) fp16, q (rows 0..33) f32;
     k/vT -> DRAM -> pair AllGather.
  3. conv3x3+BN+lrelu (CAM) f32r rows 0..33 -> g fp16 in SBUF;
     partial channel sums over rows 0..31 -> pair AllReduce.
  4. Attention over own 17 i-chunks: f32r logits, row softmax (max on
     gpsimd, exp+scale on ACT), PE transpose of prob chunks, fp16
     apply, residual fused to fp16, 1x1 out-proj -> pam_sb f32.
  5. CAM MLP (sigmoid folded into the 1x1 weights), out-proj
     accumulated into pam_sb.
  6. bilinear x2 upsample (DVE) of rows 0..32, DMA out.
"""
import sys

if '/opt/trn_rl_repo' not in sys.path:
    sys.path.insert(0, '/opt/trn_rl_repo')

import numpy as np

import concourse.bass as bass
import concourse.mybir as mybir
import concourse.tile as tile
from concourse import bacc
from concourse.bass_utils import run_bass_kernel_spmd

dt = mybir.dt
f32 = dt.float32
f32r = dt.float32r
fp16 = dt.float16
AF = mybir.ActivationFunctionType
OP = mybir.AluOpType

C = 512          # channels
P = 128          # partition size
NCH = C // P     # channel chunks (4)
H = W = 64
HW = H * W       # 4096
CR = 64          # q/k channels
OC = 64          # output channels
OWN_ROWS = 34    # rows computed per core (0..33); rows 32+ feed upsample only
OWN = OWN_ROWS * W    # 2176 = 17*128
NIC = OWN // P        # 17 attention i-chunks
KV_ROWS = 32          # rows contributed to the gathered k/v (0..31)
KOWN = KV_ROWS * W    # 2048
KJC = KOWN // P       # 16 own j-chunks
NJC = HW // P         # 32 gathered j-chunks
XR = OWN_ROWS + 1     # x rows needed (0..34: +1 halo row below)
XW = W + 2            # x cols incl zero-pad columns (f32r needs even APs)
EPS = 1e-5
RG = [[0, 1], [2, 3], [4, 5], [6, 7]]   # sample pairs

# conv h-blocks (start_row, n_rows); 7-row blocks keep the moving dim
# >= 256 even with dx-edge clamps (f32r full-rate needs >= 256)
HBLK = [(0, 7), (7, 7), (14, 7), (21, 7), (28, 6)]

# tap order: full-coverage center tap first (needed for PSUM start flag)
_ALL = [(ci, dy, dx) for ci in range(NCH) for dy in (-1, 0, 1) for dx in (-1, 0, 1)]
TAPS = [(0, 0, 0)] + [t for t in _ALL if t != (0, 0, 0)]
NT = len(TAPS)   # 36

Q_EDGES = [0, 512, 1024, 1536, 2048, OWN]
K_EDGES = [0, 512, 1024, 1536, 2048]

# attention i-blocks (first_chunk, n_chunks)
IBLK = [(0, 2), (2, 2), (4, 2), (6, 2), (8, 2), (10, 2), (12, 2), (14, 2), (16, 1)]


def _conv_tap_aps(psum_t, x_t, r0b, nr, dy, dx):
    """APs for one conv tap on block rows [r0b, r0b+nr). x_t: [128,XR,XW]
    with zero-padded columns 0 and 65 (f32r needs even-width APs)."""
    r0 = 1 if (r0b == 0 and dy == -1) else 0
    out_ap = psum_t[:, r0:nr, :]
    in_ap = x_t[:, r0b + r0 + dy: r0b + nr + dy, 1 + dx: 1 + dx + W]
    return out_ap, in_ap


def _emit_conv(nc, tc, w_d, s_ct, b_ct, x_t, dst, odt, pool_name):
    """conv3x3+BN+lrelu f32r over rows 0..33 -> dst[co] [P, OWN] (odt)."""
    with tc.tile_pool(name=pool_name, bufs=2) as pw, \
         tc.tile_pool(name=pool_name + "_ev", bufs=3) as pe, \
         tc.tile_pool(name=pool_name + "_ps", bufs=1, space="PSUM") as psc:
        for co in range(NCH):
            wt = pw.tile([P, NT, P], f32r, tag="w", name="w")
            nc.sync.dma_start(out=wt, in_=w_d[co].rearrange("t p f -> p t f"))
            for b, (r0b, nr) in enumerate(HBLK):
                pst = psc.tile([P, nr, W], f32, tag="cv", name="cv")
                for t, (ci, dy, dx) in enumerate(TAPS):
                    o_ap, i_ap = _conv_tap_aps(pst, x_t[ci], r0b, nr, dy, dx)
                    nc.tensor.matmul(o_ap, wt[:, t, :], i_ap,
                                     start=(t == 0), stop=(t == NT - 1))
                z = pe.tile([P, 7 * W], odt, tag="z", name="z")[:, 0:nr * W]
                nc.scalar.activation(
                    out=z, in_=pst.rearrange("p a b -> p (a b)"),
                    func=AF.Identity, bias=b_ct[co], scale=s_ct[co])
                nc.vector.scalar_tensor_tensor(
                    out=dst[co][:, r0b * W:(r0b + nr) * W], in0=z, scalar=0.2,
                    in1=z, op0=OP.mult, op1=OP.max)


def _emit_proj(nc, tc, d, ct, f_t, f16_t, q_t, k_own, ksc, vsc):
    """k/v own-row projections -> DRAM (gathered later), q -> SBUF."""
    with tc.tile_pool(name="proj_ev", bufs=2) as pve, \
         tc.tile_pool(name="proj_ps", bufs=2, space="PSUM") as psq:
        # k first so its AllGather starts earliest
        for bi in range(len(K_EDGES) - 1):
            off, end = K_EDGES[bi], K_EDGES[bi + 1]
            pq = psq.tile([CR, 512], f32, tag="pq", name="pq")
            for ci in range(NCH):
                nc.tensor.matmul(pq, ct['wk'][ci], f_t[ci][:, off:end],
                                 start=(ci == 0), stop=(ci == NCH - 1))
            nc.scalar.activation(out=k_own[:, off:end], in_=pq,
                                 func=AF.Identity, bias=ct['bk'], scale=1.0)
        nc.sync.dma_start(out=ksc, in_=k_own)

        for jc in range(KJC):
            pv = psq.tile([P, C], f32, tag="pv", name="pv")
            s, e = jc * P, (jc + 1) * P
            for ci in range(NCH):
                nc.tensor.matmul(pv, f_t[ci][:, s:e], ct['wv'][ci],
                                 start=(ci == 0), stop=(ci == NCH - 1))
            vtmp = pve.tile([P, C], fp16, tag="vtmp", name="vtmp")
            nc.vector.tensor_add(vtmp, pv, ct['bv'])
            nc.sync.dma_start(out=vsc[:, jc, :], in_=vtmp)

        for bi in range(len(Q_EDGES) - 1):
            off, end = Q_EDGES[bi], Q_EDGES[bi + 1]
            pq = psq.tile([CR, 512], f32, tag="pq", name="pq")[:, 0:end - off]
            for ci in range(NCH):
                nc.tensor.matmul(pq, ct['wq'][ci], f_t[ci][:, off:end],
                                 start=(ci == 0), stop=(ci == NCH - 1))
            nc.scalar.activation(out=q_t[:, off:end], in_=pq,
                                 func=AF.Identity, bias=ct['bq'], scale=1.0)


def _emit_attention(nc, tc, ct, f_t, q_t, pam_sb, kg, vg):
    with tc.tile_pool(name="kv2", bufs=1) as pv2, \
         tc.tile_pool(name="ls", bufs=2) as pls, \
         tc.tile_pool(name="e16p", bufs=2) as pep, \
         tc.tile_pool(name="et", bufs=2) as pet, \
         tc.tile_pool(name="att_tmp", bufs=2) as pat, \
         tc.tile_pool(name="res_t", bufs=2) as prs, \
         tc.tile_pool(name="ps_l", bufs=2, space="PSUM") as psl, \
         tc.tile_pool(name="ps_t", bufs=2, space="PSUM") as pstp, \
         tc.tile_pool(name="ps_a", bufs=2, space="PSUM") as psa, \
         tc.tile_pool(name="ps_p", bufs=2, space="PSUM") as psp:
        k_t = pv2.tile([CR, HW], f32r, name="k2_t", tag="k2_t")
        vt_t = pv2.tile([P, NJC, C], fp16, name="vt2_t", tag="vt2_t")
        nc.sync.dma_start(out=k_t[:, 0:KOWN], in_=kg[0])
        nc.sync.dma_start(out=k_t[:, KOWN:HW], in_=kg[1])
        nc.sync.dma_start(out=vt_t[:, 0:KJC, :], in_=vg[0])
        nc.sync.dma_start(out=vt_t[:, KJC:NJC, :], in_=vg[1])

        def emit_softmax(ic):
            """logits + row softmax for i-chunk ic -> e16 (fp16 probs)."""
            ls = pls.tile([P, HW], f32, tag="ls", name="ls")
            for jb in range(HW // 512):
                pl = psl.tile([P, 512], f32, tag="pl", name="pl")
                nc.tensor.matmul(
                    pl, q_t[:, ic * P:(ic + 1) * P],
                    k_t[:, jb * 512:(jb + 1) * 512], start=True, stop=True)
                nc.scalar.activation(
                    out=ls[:, jb * 512:(jb + 1) * 512], in_=pl,
                    func=AF.Identity, bias=0.0, scale=1.0)
            nmax = pat.tile([P, 1], f32, tag="nmax", name="nmax")
            nc.vector.tensor_reduce(out=nmax, in_=ls, axis=mybir.AxisListType.X,
                                    op=OP.max, negate=True)
            rsum = pat.tile([P, 1], f32, tag="rsum", name="rsum")
            nc.scalar.activation(out=ls, in_=ls, func=AF.Exp,
                                 bias=nmax, scale=1.0, accum_out=rsum)
            rrec = pat.tile([P, 1], f32, tag="rrec", name="rrec")
            nc.vector.reciprocal(out=rrec, in_=rsum)
            e16 = pep.tile([P, HW], fp16, tag="e16", name="e16")
            nc.scalar.activation(out=e16, in_=ls, func=AF.Identity,
                                 bias=0.0, scale=rrec)
            return e16

        def emit_transposes(e16, et_t, ph):
            for jc in range(NJC):
                pt = pstp.tile([P, P], fp16, tag="pt", name="pt")
                nc.tensor.transpose(
                    pt, e16[:, jc * P:(jc + 1) * P], ct['ident'])
                if jc % 2 == 0:
                    nc.vector.tensor_copy(
                        out=et_t[:, jc, ph * P:(ph + 1) * P], in_=pt)
                else:
                    nc.scalar.activation(
                        out=et_t[:, jc, ph * P:(ph + 1) * P], in_=pt,
                        func=AF.Identity, bias=0.0, scale=1.0)

        # software pipeline: block b's apply covers block b+1's softmax chain
        e16s = {ic: emit_softmax(ic) for ic in (0, 1)}
        for ib, (ic0, nch) in enumerate(IBLK):
            isz = P * nch
            ioff = ic0 * P
            et_t = pet.tile([P, NJC, 2 * P], fp16, tag="et", name="et")
            for ph in range(nch):
                emit_transposes(e16s.pop(ic0 + ph), et_t, ph)
            if ib + 1 < len(IBLK):
                nic0, nnch = IBLK[ib + 1]
                for ph in range(nnch):
                    e16s[nic0 + ph] = emit_softmax(nic0 + ph)
            r16 = []
            for co in range(NCH):
                pa = psa.tile([P, 2 * P], f32, tag="pa", name="pa")[:, 0:isz]
                for jc in range(NJC):
                    nc.tensor.matmul(
                        pa, vt_t[:, jc, co * P:(co + 1) * P],
                        et_t[:, jc, 0:isz],
                        start=(jc == 0), stop=(jc == NJC - 1))
                rt = prs.tile([P, 2 * P], fp16, tag=f"r{co}", name=f"r{co}")[:, 0:isz]
                nc.vector.scalar_tensor_tensor(
                    out=rt, in0=pa, scalar=ct['alpha'],
                    in1=f_t[co][:, ioff:ioff + isz], op0=OP.mult, op1=OP.add)
                r16.append(rt)
            pp = psp.tile([OC, 2 * P], f32, tag="pp", name="pp")[:, 0:isz]
            for ci in range(NCH):
                nc.tensor.matmul(pp, ct['wpo'][ci], r16[ci],
                                 start=(ci == 0), stop=(ci == NCH - 1))
            nc.scalar.activation(out=pam_sb[:, ioff:ioff + isz], in_=pp,
                                 func=AF.Identity, bias=ct['bpo'], scale=1.0)


def _emit_cam_tail(nc, tc, d, ct, g_t, pam_sb, msg):
    """channel-attention MLP (mean already AllReduced into msg) + 1x1 out."""
    with tc.tile_pool(name="mlp", bufs=1) as pm, \
         tc.tile_pool(name="cam_ev", bufs=2) as pce, \
         tc.tile_pool(name="ps_mlp", bufs=2, space="PSUM") as psm, \
         tc.tile_pool(name="ps_co", bufs=2, space="PSUM") as psco:
        msum = [pm.tile([P, 1], f32, name=f"ms{i}", tag=f"ms{i}") for i in range(NCH)]
        for i in range(NCH):
            nc.sync.dma_start(out=msum[i], in_=msg[i])
        wc1_t = [pm.tile([P, CR], f32, name=f"w1{i}", tag=f"w1{i}") for i in range(NCH)]
        wc2_t = [pm.tile([CR, P], f32, name=f"w2{i}", tag=f"w2{i}") for i in range(NCH)]
        wco_t = [pm.tile([P, OC], f32, name=f"wo{i}", tag=f"wo{i}") for i in range(NCH)]
        bc2_t = [pm.tile([P, 1], f32, name=f"b2{i}", tag=f"b2{i}") for i in range(NCH)]
        for i in range(NCH):
            nc.sync.dma_start(out=wc1_t[i], in_=d['wc1'][i])
            nc.sync.dma_start(out=wc2_t[i], in_=d['wc2'][i])
            nc.sync.dma_start(out=wco_t[i], in_=d['wco'][i])
            nc.sync.dma_start(out=bc2_t[i], in_=d['bc2'][i])
        p1 = psm.tile([CR, 1], f32, tag="p1", name="p1")
        for ci in range(NCH):
            nc.tensor.matmul(p1, wc1_t[ci], msum[ci],
                             start=(ci == 0), stop=(ci == NCH - 1))
        t1 = pm.tile([CR, 1], f32, name="t1", tag="t1")
        nc.scalar.activation(out=t1, in_=p1, func=AF.Identity,
                             bias=ct['bc1'], scale=1.0)
        y1 = pm.tile([CR, 1], f32, name="y1", tag="y1")
        nc.vector.scalar_tensor_tensor(out=y1, in0=t1, scalar=0.2, in1=t1,
                                       op0=OP.mult, op1=OP.max)
        wce = [pm.tile([P, OC], fp16, name=f"we{i}", tag=f"we{i}") for i in range(NCH)]
        for co in range(NCH):
            p2 = psm.tile([P, 1], f32, tag="p2", name="p2")
            nc.tensor.matmul(p2, wc2_t[co], y1, start=True, stop=True)
            s_t = pm.tile([P, 1], f32, name=f"s{co}", tag=f"s{co}")
            nc.scalar.activation(out=s_t, in_=p2, func=AF.Sigmoid,
                                 bias=bc2_t[co], scale=1.0)
            nc.vector.tensor_scalar_mul(wce[co], wco_t[co], s_t)
        for bi in range(len(Q_EDGES) - 1):
            off, end = Q_EDGES[bi], Q_EDGES[bi + 1]
            sz = end - off
            pco = psco.tile([OC, 512], f32, tag="pco", name="pco")[:, 0:sz]
            for ci in range(NCH):
                nc.tensor.matmul(pco, wce[ci], g_t[ci][:, off:end],
                                 start=(ci == 0), stop=(ci == NCH - 1))
            zc = pce.tile([OC, 512], f32, tag="zc", name="zc")[:, 0:sz]
            nc.scalar.activation(out=zc, in_=pco, func=AF.Identity,
                                 bias=ct['bco'], scale=1.0)
            nc.vector.tensor_add(pam_sb[:, off:end], pam_sb[:, off:end], zc)


def _emit_upsample(nc, tc, pam_sb, y_d):
    with tc.tile_pool(name="up", bufs=1) as pu:
        su = pam_sb.rearrange("p (a b) -> p a b", b=W)  # [OC,34,64]
        a_t = pu.tile([OC, OWN_ROWS, W], f32, name="a_t", tag="a_t")
        b_t = pu.tile([OC, OWN_ROWS, W], f32, name="b_t", tag="b_t")
        nc.scalar.activation(out=a_t.rearrange("p a b -> p (a b)"), in_=pam_sb,
                             func=AF.Identity, bias=0.0, scale=0.75)
        nc.scalar.activation(out=b_t.rearrange("p a b -> p (a b)"), in_=pam_sb,
                             func=AF.Identity, bias=0.0, scale=0.25)
        sh = pu.tile([OC, OWN_ROWS, W, 2], f32, name="sh", tag="sh")
        nc.vector.tensor_copy(out=sh[:, :, 0, 0], in_=su[:, :, 0])
        nc.vector.tensor_add(sh[:, :, 1:W, 0], b_t[:, :, 0:W - 1], a_t[:, :, 1:W])
        nc.vector.tensor_add(sh[:, :, 0:W - 1, 1], a_t[:, :, 0:W - 1], b_t[:, :, 1:W])
        nc.vector.tensor_copy(out=sh[:, :, W - 1, 1], in_=su[:, :, W - 1])
        au = pu.tile([OC, OWN_ROWS, 2 * W], f32, name="au", tag="au")
        bu = pu.tile([OC, OWN_ROWS, 2 * W], f32, name="bu", tag="bu")
        shf = sh.rearrange("p a b c -> p a (b c)")
        nc.scalar.activation(out=au.rearrange("p a b -> p (a b)"),
                             in_=shf.rearrange("p a b -> p (a b)"),
                             func=AF.Identity, bias=0.0, scale=0.75)
        nc.scalar.activation(out=bu.rearrange("p a b -> p (a b)"),
                             in_=shf.rearrange("p a b -> p (a b)"),
                             func=AF.Identity, bias=0.0, scale=0.25)
        out_t = pu.tile([OC, H // 2, 2, 2 * W], f32, name="out_t", tag="out_t")
        nc.vector.tensor_copy(out=out_t[:, 0, 0, :], in_=shf[:, 0, :])
        nc.vector.tensor_add(out_t[:, 1:H // 2, 0, :], bu[:, 0:H // 2 - 1, :],
                             au[:, 1:H // 2, :])
        nc.vector.tensor_add(out_t[:, 0:H // 2, 1, :], au[:, 0:H // 2, :],
                             bu[:, 1:H // 2 + 1, :])
        nc.sync.dma_start(out=y_d, in_=out_t.rearrange("p a b c -> p (a b) c"))


SOLO = False  # timing experiment: replace collectives with local DMAs


def _build():
    nc = bacc.Bacc("TRN2", target_bir_lowering=False, debug=False,
                   enable_asserts=True, num_devices=8)

    def din(name, shape, dtp=f32):
        return nc.dram_tensor(name, shape, dtp, kind="ExternalInput").ap()

    d = {
        'xs': din("xs", [NCH, P, XR, XW], f32r),
        'wpm': din("wpm", [NCH, NT, P, P], f32r),
        'wcm': din("wcm", [NCH, NT, P, P], f32r),
        'sp': din("sp", [NCH, P, 1]), 'bp': din("bp", [NCH, P, 1]),
        'sc': din("sc", [NCH, P, 1]), 'bc': din("bc", [NCH, P, 1]),
        'wq': din("wq", [NCH, P, CR], f32r), 'wk': din("wk", [NCH, P, CR], f32r),
        'bq': din("bq", [CR, 1]), 'bk': din("bk", [CR, 1]),
        'wv': din("wv", [NCH, P, C], f32r), 'bv': din("bv", [1, C]),
        'alpha': din("alpha", [1, 1]),
        'wpo': din("wpo", [NCH, P, OC], fp16), 'bpo': din("bpo", [OC, 1]),
        'wc1': din("wc1", [NCH, P, CR]), 'bc1': din("bc1", [CR, 1]),
        'wc2': din("wc2", [NCH, CR, P]), 'bc2': din("bc2", [NCH, P, 1]),
        'wco': din("wco", [NCH, P, OC]), 'bco': din("bco", [OC, 1]),
        'ident': din("ident", [P, P], fp16),
    }
    y_d = nc.dram_tensor("y", [OC, H, 2 * W], f32, kind="ExternalOutput").ap()

    with tile.TileContext(nc) as tc:
        with tc.tile_pool(name="consts", bufs=1) as pc, \
             tc.tile_pool(name="fdram", bufs=1, space="DRAM") as pfd:
            ct = {}
            ct['ident'] = pc.tile([P, P], fp16, name="ident", tag="ident")
            nc.sync.dma_start(out=ct['ident'], in_=d['ident'])
            ct['alpha'] = pc.tile([P, 1], f32, name="alpha_t", tag="alpha_t")
            nc.sync.dma_start(out=ct['alpha'], in_=d['alpha'].to_broadcast([P, 1]))
            ct['bv'] = pc.tile([P, C], f32, name="bv_t", tag="bv_t")
            nc.sync.dma_start(out=ct['bv'], in_=d['bv'].to_broadcast([P, C]))
            for nm, rows in (('bq', CR), ('bk', CR), ('bpo', OC), ('bco', OC),
                             ('bc1', CR)):
                ct[nm] = pc.tile([rows, 1], f32, name=f"bias_{nm}", tag=f"bias_{nm}")
                nc.sync.dma_start(out=ct[nm], in_=d[nm])
            for nm in ('sp', 'bp', 'sc', 'bc'):
                ct[nm] = [pc.tile([P, 1], f32, name=f"{nm}_{i}_t", tag=f"{nm}_{i}_t")
                          for i in range(NCH)]
                for i in range(NCH):
                    nc.sync.dma_start(out=ct[nm][i], in_=d[nm][i])
            for nm, fr, dtp in (('wq', CR, f32r), ('wk', CR, f32r),
                                ('wv', C, f32r), ('wpo', OC, fp16)):
                ct[nm] = [pc.tile([P, fr], dtp, name=f"{nm}{i}_t", tag=f"{nm}{i}_t")
                          for i in range(NCH)]
                for i in range(NCH):
                    nc.sync.dma_start(out=ct[nm][i], in_=d[nm][i])

            ksc = pfd.tile([CR, KOWN], f32r, name="ksc", tag="ksc")
            kg = pfd.tile([2, CR, KOWN], f32r, name="kg", tag="kg")
            vsc = pfd.tile([P, KJC, C], fp16, name="vsc", tag="vsc")
            vg = pfd.tile([2, P, KJC, C], fp16, name="vg", tag="vg")
            msc = pfd.tile([NCH, P, 1], f32, name="msc", tag="msc")
            msg = pfd.tile([NCH, P, 1], f32, name="msg", tag="msg")

            with tc.tile_pool(name="f16_store", bufs=1) as p_f16, \
                 tc.tile_pool(name="g_store", bufs=1) as p_g, \
                 tc.tile_pool(name="qk_sb", bufs=1) as p_qk, \
                 tc.tile_pool(name="pam_out", bufs=1) as p_pam:
                f16_t = [p_f16.tile([P, OWN], fp16, name=f"h{i}", tag=f"h{i}")
                         for i in range(NCH)]
                g_t = [p_g.tile([P, OWN], fp16, name=f"g{i}", tag=f"g{i}")
                       for i in range(NCH)]
                q_t = p_qk.tile([CR, OWN], f32r, name="q_t", tag="q_t")
                pam_sb = p_pam.tile([OC, OWN], f32, name="pam_sb", tag="pam_sb")

                with tc.tile_pool(name="xs_sb", bufs=1) as px, \
                     tc.tile_pool(name="f_store", bufs=1) as p_f:
                    x_t = [px.tile([P, XR, XW], f32r, name=f"x{i}", tag=f"x{i}")
                           for i in range(NCH)]
                    f_t = [p_f.tile([P, OWN], f32r, name=f"f{i}", tag=f"f{i}")
                           for i in range(NCH)]
                    k_own = p_f.tile([CR, KOWN], f32r, name="k_own", tag="k_own")
                    for i in range(NCH):
                        nc.sync.dma_start(out=x_t[i], in_=d['xs'][i])
                    _emit_conv(nc, tc, d['wpm'], ct['sp'], ct['bp'], x_t, f_t,
                               f32, "pamconv")
                    _emit_proj(nc, tc, d, ct, f_t, f16_t, q_t, k_own, ksc, vsc)
                    for i in range(NCH):
                        nc.vector.tensor_copy(out=f16_t[i], in_=f_t[i])
                    if SOLO:
                        nc.sync.dma_start(out=kg[0], in_=ksc)
                        nc.sync.dma_start(out=kg[1], in_=ksc)
                        nc.sync.dma_start(out=vg[0], in_=vsc)
                        nc.sync.dma_start(out=vg[1], in_=vsc)
                    else:
                        nc.gpsimd.collective_compute(
                            "AllGather", OP.bypass, replica_groups=RG,
                            ins=[ksc.opt()], outs=[kg.opt()])
                        nc.gpsimd.collective_compute(
                            "AllGather", OP.bypass, replica_groups=RG,
                            ins=[vsc.opt()], outs=[vg.opt()])
                    _emit_conv(nc, tc, d['wcm'], ct['sc'], ct['bc'], x_t, g_t,
                               fp16, "camconv")
                    with tc.tile_pool(name="msump", bufs=1) as pms:
                        for i in range(NCH):
                            ms = pms.tile([P, 1], f32, name=f"pm{i}", tag=f"pm{i}")
                            nc.vector.tensor_reduce(
                                out=ms, in_=g_t[i][:, 0:KOWN],
                                axis=mybir.AxisListType.X, op=OP.add)
                            nc.sync.dma_start(out=msc[i], in_=ms)
                        if SOLO:
                            nc.sync.dma_start(out=msg, in_=msc)
                        else:
                            nc.gpsimd.collective_compute(
                                "AllReduce", OP.add, replica_groups=RG,
                                ins=[msc.opt()], outs=[msg.opt()])

                _emit_attention(nc, tc, ct, f16_t, q_t, pam_sb, kg, vg)
                _emit_cam_tail(nc, tc, d, ct, g_t, pam_sb, msg)
                _emit_upsample(nc, tc, pam_sb, y_d)
    nc.compile()
    return nc


_NC_CACHE = None


def _get_nc():
    global _NC_CACHE
    if _NC_CACHE is None:
        _NC_CACHE = _build()
    return _NC_CACHE


_TAP_CI = np.array([t[0] for t in TAPS])
_TAP_DY = np.array([t[1] + 1 for t in TAPS])
_TAP_DX = np.array([t[2] + 1 for t in TAPS])


def _pack_conv(wfull):
    """[C, C, 3, 3] -> [NCH(co), NT, P(ci_local), P(co_local)] lhsT tiles."""
    wr = np.asarray(wfull, np.float32).reshape(NCH, P, NCH, P, 3, 3)
    wt = wr.transpose(0, 2, 4, 5, 3, 1)  # [co, ci, dy, dx, ci_l, co_l]
    return np.ascontiguousarray(wt[:, _TAP_CI, _TAP_DY, _TAP_DX])


def _packT(w, free):
    """w [free, C] -> [NCH, P, free] lhsT chunks."""
    return np.ascontiguousarray(np.asarray(w, np.float32).T.reshape(NCH, P, free))


def _prep_shared(inputs, flip):
    wp = np.asarray(inputs['W_pam_in'], np.float32)
    wc = np.asarray(inputs['W_cam_in'], np.float32)
    if flip:
        wp = wp[:, :, ::-1, :]
        wc = wc[:, :, ::-1, :]

    def bnfold(g, b, m, v):
        s = (np.asarray(g, np.float32)
             / np.sqrt(np.asarray(v, np.float32) + EPS)).astype(np.float32)
        bb = (np.asarray(b, np.float32)
              - np.asarray(m, np.float32) * s).astype(np.float32)
        return s.reshape(NCH, P, 1), bb.reshape(NCH, P, 1)

    sp, bp = bnfold(inputs['pam_gamma'], inputs['pam_beta'],
                    inputs['pam_mean'], inputs['pam_var'])
    sc, bc = bnfold(inputs['cam_gamma'], inputs['cam_beta'],
                    inputs['cam_mean'], inputs['cam_var'])
    wc2 = np.ascontiguousarray(
        np.asarray(inputs['Wc2'], np.float32).reshape(NCH, P, CR).transpose(0, 2, 1))
    return {
        'wpm': _pack_conv(wp),
        'wcm': _pack_conv(wc),
        'sp': sp, 'bp': bp, 'sc': sc, 'bc': bc,
        'wq': _packT(inputs['Wq'], CR),
        'wk': _packT(inputs['Wk'], CR),
        'bq': np.asarray(inputs['bq'], np.float32).reshape(CR, 1),
        'bk': np.asarray(inputs['bk'], np.float32).reshape(CR, 1),
        'wv': _packT(inputs['Wv'], C),
        'bv': np.asarray(inputs['bv'], np.float32).reshape(1, C),
        'alpha': np.asarray(inputs['alpha'], np.float32).reshape(1, 1),
        'wpo': _packT(inputs['W_pam_out'], OC).astype(np.float16),
        'bpo': np.asarray(inputs['b_pam_out'], np.float32).reshape(OC, 1),
        'wc1': _packT(np.asarray(inputs['Wc1'], np.float32) / HW, CR),
        'bc1': np.asarray(inputs['bc1'], np.float32).reshape(CR, 1),
        'wc2': wc2,
        'bc2': np.asarray(inputs['bc2'], np.float32).reshape(NCH, P, 1),
        'wco': _packT(inputs['W_cam_out'], OC),
        'bco': np.asarray(inputs['b_cam_out'], np.float32).reshape(OC, 1),
        'ident': np.eye(P, dtype=np.float16),
    }


def _make_in_maps(inputs):
    x = np.asarray(inputs['x'], np.float32)  # [4, 512, 64, 64]
    shared = {f: _prep_shared(inputs, f) for f in (False, True)}
    in_maps = []
    for c in range(8):
        s, flip = c // 2, c % 2
        xs = x[s]
        if flip:
            xs = xs[:, ::-1, :]
        xp = np.zeros((C, XR, XW), np.float32)
        xp[:, :, 1:1 + W] = xs[:, 0:XR, :]
        m = dict(shared[bool(flip)])
        m['xs'] = np.ascontiguousarray(xp.reshape(NCH, P, XR, XW))
        in_maps.append(m)
    return in_maps


def kernel(**inputs):
    nc = _get_nc()
    in_maps = _make_in_maps(inputs)
    res = run_bass_kernel_spmd(nc, in_maps, list(range(8)))
    out = np.empty((4, OC, 2 * H, 2 * W), np.float32)
    for c in range(8):
        s, flip = c // 2, c % 2
        o = res.results[c]['y']  # [64, 64, 128]
        if flip:
            out[s, :, H:2 * H, :] = o[:, ::-1, :]
        else:
            out[s, :, 0:H, :] = o
    return out


# revision 23
# speedup vs baseline: 2.6158x; 1.2062x over previous
"""DAHead (dual-attention head) Trainium2 kernel, v2.

8-core SPMD with pair collectives: core c handles sample c//2, spatial
half c%2. Odd cores see the sample vertically flipped (conv weights
flipped along dy) so every core runs the same program; the host
un-flips the output half.

v2 vs v1: the conv3x3 / q/k/v work is split across the sample's core
PAIR: each core computes f and g only on its own 34 rows, k/v only on
its own 32 rows, and the pair AllGathers k (f32, 0.5MB) and vT (fp16,
2MB) plus AllReduces the CAM channel-mean, overlapped with the CAM
conv. Attention j-order becomes [pair-rank0 rows, pair-rank1 rows],
a permutation of the 4096 tokens - softmax+apply are
permutation-invariant over j.

Precision: the softmax is argmax-like (logits up to ~375), so the
q/k path runs in f32r (full-rate PE mode, ~1.6e-4 matmul error vs
fp16's 2.9e-4): PAM conv f32r -> f kept f32 in SBUF -> q/k f32r ->
logits f32r. The linear paths (v, attention apply, residual,
out-projs, whole CAM branch) are fp16/f32r single-pass.

Per-core program:
  1. conv3x3+BN+lrelu (PAM) f32r, rows 0..33 -> f f32 in SBUF.
  2. k (rows 0..31) f32, vT (rows 0..31) fp16, q (rows 0..33) f32;
     k/vT -> DRAM -> pair AllGather.
  3. conv3x3+BN+lrelu (CAM) f32r rows 0..33 -> g fp16 in SBUF;
     partial channel sums over rows 0..31 -> pair AllReduce.
  4. Attention over own 17 i-chunks: f32r logits, row softmax (max on
     gpsimd, exp+scale on ACT), PE transpose of prob chunks, fp16
     apply, residual fused to fp16, 1x1 out-proj -> pam_sb f32.
  5. CAM MLP (sigmoid folded into the 1x1 weights), out-proj
     accumulated into pam_sb.
  6. bilinear x2 upsample (DVE) of rows 0..32, DMA out.
"""
import sys

if '/opt/trn_rl_repo' not in sys.path:
    sys.path.insert(0, '/opt/trn_rl_repo')

import numpy as np

import concourse.bass as bass
import concourse.mybir as mybir
import concourse.tile as tile
from concourse import bacc
from concourse.bass_utils import run_bass_kernel_spmd

dt = mybir.dt
f32 = dt.float32
f32r = dt.float32r
fp16 = dt.float16
AF = mybir.ActivationFunctionType
OP = mybir.AluOpType

C = 512          # channels
P = 128          # partition size
NCH = C // P     # channel chunks (4)
H = W = 64
HW = H * W       # 4096
CR = 64          # q/k channels
OC = 64          # output channels
OWN_ROWS = 34    # rows computed per core (0..33); rows 32+ feed upsample only
OWN = OWN_ROWS * W    # 2176 = 17*128
NIC = OWN // P        # 17 attention i-chunks
KV_ROWS = 32          # rows contributed to the gathered k/v (0..31)
KOWN = KV_ROWS * W    # 2048
KJC = KOWN // P       # 16 own j-chunks
NJC = HW // P         # 32 gathered j-chunks
XR = OWN_ROWS + 1     # x rows needed (0..34: +1 halo row below)
XW = W + 2            # x cols incl zero-pad columns (f32r needs even APs)
EPS = 1e-5
RG = [[0, 1], [2, 3], [4, 5], [6, 7]]   # sample pairs

# conv h-blocks (start_row, n_rows); 7-row blocks keep the moving dim
# >= 256 even with dx-edge clamps (f32r full-rate needs >= 256)
HBLK = [(0, 7), (7, 7), (14, 7), (21, 7), (28, 6)]

# tap order: full-coverage center tap first (needed for PSUM start flag)
_ALL = [(ci, dy, dx) for ci in range(NCH) for dy in (-1, 0, 1) for dx in (-1, 0, 1)]
TAPS = [(0, 0, 0)] + [t for t in _ALL if t != (0, 0, 0)]
NT = len(TAPS)   # 36

Q_EDGES = [0, 512, 1024, 1536, 2048, OWN]
K_EDGES = [0, 512, 1024, 1536, 2048]

# attention i-blocks (first_chunk, n_chunks)
IBLK = [(0, 2), (2, 2), (4, 2), (6, 2), (8, 2), (10, 2), (12, 2), (14, 2), (16, 1)]


def _conv_tap_aps(psum_t, x_t, r0b, nr, dy, dx):
    """APs for one conv tap on block rows [r0b, r0b+nr). x_t: [128,XR,XW]
    with zero-padded columns 0 and 65 (f32r needs even-width APs)."""
    r0 = 1 if (r0b == 0 and dy == -1) else 0
    out_ap = psum_t[:, r0:nr, :]
    in_ap = x_t[:, r0b + r0 + dy: r0b + nr + dy, 1 + dx: 1 + dx + W]
    return out_ap, in_ap


def _emit_conv(nc, tc, w_d, s_ct, b_ct, x_t, dst, odt, pool_name, pw, wt0):
    """conv3x3+BN+lrelu f32r over rows 0..33 -> dst[co] [P, OWN] (odt)."""
    with tc.tile_pool(name=pool_name + "_ev", bufs=3) as pe, \
         tc.tile_pool(name=pool_name + "_ps", bufs=1, space="PSUM") as psc:
        for co in range(NCH):
            if co == 0:
                wt = wt0
            else:
                wt = pw.tile([P, NT, P], f32r, tag="w", name="w")
                nc.scalar.dma_start(out=wt, in_=w_d[co].rearrange("t p f -> p t f"))
            for b, (r0b, nr) in enumerate(HBLK):
                pst = psc.tile([P, nr, W], f32, tag="cv", name="cv")
                for t, (ci, dy, dx) in enumerate(TAPS):
                    o_ap, i_ap = _conv_tap_aps(pst, x_t[ci], r0b, nr, dy, dx)
                    nc.tensor.matmul(o_ap, wt[:, t, :], i_ap,
                                     start=(t == 0), stop=(t == NT - 1))
                z = pe.tile([P, 7 * W], odt, tag="z", name="z")[:, 0:nr * W]
                nc.scalar.activation(
                    out=z, in_=pst.rearrange("p a b -> p (a b)"),
                    func=AF.Identity, bias=b_ct[co], scale=s_ct[co])
                nc.vector.scalar_tensor_tensor(
                    out=dst[co][:, r0b * W:(r0b + nr) * W], in0=z, scalar=0.2,
                    in1=z, op0=OP.mult, op1=OP.max)


def _emit_proj(nc, tc, d, ct, f_t, f16_t, q_t, k_own, ksc, vsc):
    """k/v own-row projections -> DRAM (gathered later), q -> SBUF."""
    with tc.tile_pool(name="proj_ev", bufs=2) as pve, \
         tc.tile_pool(name="proj_ps", bufs=2, space="PSUM") as psq:
        # k first so its AllGather starts earliest
        for bi in range(len(K_EDGES) - 1):
            off, end = K_EDGES[bi], K_EDGES[bi + 1]
            pq = psq.tile([CR, 512], f32, tag="pq", name="pq")
            for ci in range(NCH):
                nc.tensor.matmul(pq, ct['wk'][ci], f_t[ci][:, off:end],
                                 start=(ci == 0), stop=(ci == NCH - 1))
            nc.scalar.activation(out=k_own[:, off:end], in_=pq,
                                 func=AF.Identity, bias=ct['bk'], scale=1.0)
        nc.sync.dma_start(out=ksc, in_=k_own)

        for jc in range(KJC):
            pv = psq.tile([P, C], f32, tag="pv", name="pv")
            s, e = jc * P, (jc + 1) * P
            for ci in range(NCH):
                nc.tensor.matmul(pv, f_t[ci][:, s:e], ct['wv'][ci],
                                 start=(ci == 0), stop=(ci == NCH - 1))
            vtmp = pve.tile([P, C], fp16, tag="vtmp", name="vtmp")
            nc.vector.tensor_add(vtmp, pv, ct['bv'])
            nc.sync.dma_start(out=vsc[:, jc, :], in_=vtmp)

        for bi in range(len(Q_EDGES) - 1):
            off, end = Q_EDGES[bi], Q_EDGES[bi + 1]
            pq = psq.tile([CR, 512], f32, tag="pq", name="pq")[:, 0:end - off]
            for ci in range(NCH):
                nc.tensor.matmul(pq, ct['wq'][ci], f_t[ci][:, off:end],
                                 start=(ci == 0), stop=(ci == NCH - 1))
            nc.scalar.activation(out=q_t[:, off:end], in_=pq,
                                 func=AF.Identity, bias=ct['bq'], scale=1.0)


def _emit_attention(nc, tc, ct, f_t, q_t, k_t, pam_sb, vg):
    with tc.tile_pool(name="kv2", bufs=1) as pv2, \
         tc.tile_pool(name="ls", bufs=2) as pls, \
         tc.tile_pool(name="e16p", bufs=2) as pep, \
         tc.tile_pool(name="et", bufs=2) as pet, \
         tc.tile_pool(name="att_tmp", bufs=2) as pat, \
         tc.tile_pool(name="res_t", bufs=2) as prs, \
         tc.tile_pool(name="ps_l", bufs=2, space="PSUM") as psl, \
         tc.tile_pool(name="ps_t", bufs=2, space="PSUM") as pstp, \
         tc.tile_pool(name="ps_a", bufs=2, space="PSUM") as psa, \
         tc.tile_pool(name="ps_p", bufs=2, space="PSUM") as psp:
        vt_t = pv2.tile([P, NJC, C], fp16, name="vt2_t", tag="vt2_t")
        nc.sync.dma_start(out=vt_t[:, 0:KJC, :], in_=vg[0])
        nc.sync.dma_start(out=vt_t[:, KJC:NJC, :], in_=vg[1])

        def emit_softmax(ic):
            """logits + row softmax for i-chunk ic -> e16 (fp16 probs)."""
            ls = pls.tile([P, HW], f32, tag="ls", name="ls")
            for jb in range(HW // 512):
                pl = psl.tile([P, 512], f32, tag="pl", name="pl")
                nc.tensor.matmul(
                    pl, q_t[:, ic * P:(ic + 1) * P],
                    k_t[:, jb * 512:(jb + 1) * 512], start=True, stop=True)
                nc.scalar.activation(
                    out=ls[:, jb * 512:(jb + 1) * 512], in_=pl,
                    func=AF.Identity, bias=0.0, scale=1.0)
            nmax = pat.tile([P, 1], f32, tag="nmax", name="nmax")
            nc.vector.tensor_reduce(out=nmax, in_=ls, axis=mybir.AxisListType.X,
                                    op=OP.max, negate=True)
            rsum = pat.tile([P, 1], f32, tag="rsum", name="rsum")
            nc.scalar.activation(out=ls, in_=ls, func=AF.Exp,
                                 bias=nmax, scale=1.0, accum_out=rsum)
            rrec = pat.tile([P, 1], f32, tag="rrec", name="rrec")
            nc.vector.reciprocal(out=rrec, in_=rsum)
            e16 = pep.tile([P, HW], fp16, tag="e16", name="e16")
            nc.scalar.activation(out=e16, in_=ls, func=AF.Identity,
                                 bias=0.0, scale=rrec)
            return e16

        def emit_transposes(e16, et_t, ph):
            for jc in range(NJC):
                pt = pstp.tile([P, P], fp16, tag="pt", name="pt")
                nc.tensor.transpose(
                    pt, e16[:, jc * P:(jc + 1) * P], ct['ident'])
                if jc % 2 == 0:
                    nc.vector.tensor_copy(
                        out=et_t[:, jc, ph * P:(ph + 1) * P], in_=pt)
                else:
                    nc.scalar.activation(
                        out=et_t[:, jc, ph * P:(ph + 1) * P], in_=pt,
                        func=AF.Identity, bias=0.0, scale=1.0)

        # software pipeline: block b's apply covers block b+1's softmax chain
        e16s = {ic: emit_softmax(ic) for ic in (0, 1)}
        for ib, (ic0, nch) in enumerate(IBLK):
            isz = P * nch
            ioff = ic0 * P
            et_t = pet.tile([P, NJC, 2 * P], fp16, tag="et", name="et")
            for ph in range(nch):
                emit_transposes(e16s.pop(ic0 + ph), et_t, ph)
            if ib + 1 < len(IBLK):
                nic0, nnch = IBLK[ib + 1]
                for ph in range(nnch):
                    e16s[nic0 + ph] = emit_softmax(nic0 + ph)
            r16 = []
            for co in range(NCH):
                pa = psa.tile([P, 2 * P], f32, tag="pa", name="pa")[:, 0:isz]
                for jc in range(NJC):
                    nc.tensor.matmul(
                        pa, vt_t[:, jc, co * P:(co + 1) * P],
                        et_t[:, jc, 0:isz],
                        start=(jc == 0), stop=(jc == NJC - 1))
                rt = prs.tile([P, 2 * P], fp16, tag=f"r{co}", name=f"r{co}")[:, 0:isz]
                nc.vector.scalar_tensor_tensor(
                    out=rt, in0=pa, scalar=ct['alpha'],
                    in1=f_t[co][:, ioff:ioff + isz], op0=OP.mult, op1=OP.add)
                r16.append(rt)
            pp = psp.tile([OC, 2 * P], f32, tag="pp", name="pp")[:, 0:isz]
            for ci in range(NCH):
                nc.tensor.matmul(pp, ct['wpo'][ci], r16[ci],
                                 start=(ci == 0), stop=(ci == NCH - 1))
            nc.scalar.activation(out=pam_sb[:, ioff:ioff + isz], in_=pp,
                                 func=AF.Identity, bias=ct['bpo'], scale=1.0)


def _emit_cam_tail(nc, tc, d, ct, g_t, pam_sb, msg):
    """channel-attention MLP (mean already AllReduced into msg) + 1x1 out."""
    with tc.tile_pool(name="mlp", bufs=1) as pm, \
         tc.tile_pool(name="cam_ev", bufs=2) as pce, \
         tc.tile_pool(name="ps_mlp", bufs=2, space="PSUM") as psm, \
         tc.tile_pool(name="ps_co", bufs=2, space="PSUM") as psco:
        msum = [pm.tile([P, 1], f32, name=f"ms{i}", tag=f"ms{i}") for i in range(NCH)]
        for i in range(NCH):
            nc.sync.dma_start(out=msum[i], in_=msg[i])
        wc1_t = [pm.tile([P, CR], f32, name=f"w1{i}", tag=f"w1{i}") for i in range(NCH)]
        wc2_t = [pm.tile([CR, P], f32, name=f"w2{i}", tag=f"w2{i}") for i in range(NCH)]
        wco_t = [pm.tile([P, OC], f32, name=f"wo{i}", tag=f"wo{i}") for i in range(NCH)]
        bc2_t = [pm.tile([P, 1], f32, name=f"b2{i}", tag=f"b2{i}") for i in range(NCH)]
        for i in range(NCH):
            nc.sync.dma_start(out=wc1_t[i], in_=d['wc1'][i])
            nc.sync.dma_start(out=wc2_t[i], in_=d['wc2'][i])
            nc.sync.dma_start(out=wco_t[i], in_=d['wco'][i])
            nc.sync.dma_start(out=bc2_t[i], in_=d['bc2'][i])
        p1 = psm.tile([CR, 1], f32, tag="p1", name="p1")
        for ci in range(NCH):
            nc.tensor.matmul(p1, wc1_t[ci], msum[ci],
                             start=(ci == 0), stop=(ci == NCH - 1))
        t1 = pm.tile([CR, 1], f32, name="t1", tag="t1")
        nc.scalar.activation(out=t1, in_=p1, func=AF.Identity,
                             bias=ct['bc1'], scale=1.0)
        y1 = pm.tile([CR, 1], f32, name="y1", tag="y1")
        nc.vector.scalar_tensor_tensor(out=y1, in0=t1, scalar=0.2, in1=t1,
                                       op0=OP.mult, op1=OP.max)
        wce = [pm.tile([P, OC], fp16, name=f"we{i}", tag=f"we{i}") for i in range(NCH)]
        for co in range(NCH):
            p2 = psm.tile([P, 1], f32, tag="p2", name="p2")
            nc.tensor.matmul(p2, wc2_t[co], y1, start=True, stop=True)
            s_t = pm.tile([P, 1], f32, name=f"s{co}", tag=f"s{co}")
            nc.scalar.activation(out=s_t, in_=p2, func=AF.Sigmoid,
                                 bias=bc2_t[co], scale=1.0)
            nc.vector.tensor_scalar_mul(wce[co], wco_t[co], s_t)
        for bi in range(len(Q_EDGES) - 1):
            off, end = Q_EDGES[bi], Q_EDGES[bi + 1]
            sz = end - off
            pco = psco.tile([OC, 512], f32, tag="pco", name="pco")[:, 0:sz]
            for ci in range(NCH):
                nc.tensor.matmul(pco, wce[ci], g_t[ci][:, off:end],
                                 start=(ci == 0), stop=(ci == NCH - 1))
            zc = pce.tile([OC, 512], f32, tag="zc", name="zc")[:, 0:sz]
            nc.scalar.activation(out=zc, in_=pco, func=AF.Identity,
                                 bias=ct['bco'], scale=1.0)
            nc.vector.tensor_add(pam_sb[:, off:end], pam_sb[:, off:end], zc)


def _emit_upsample(nc, tc, pam_sb, y_d):
    with tc.tile_pool(name="up", bufs=1) as pu:
        su = pam_sb.rearrange("p (a b) -> p a b", b=W)  # [OC,34,64]
        a_t = pu.tile([OC, OWN_ROWS, W], f32, name="a_t", tag="a_t")
        b_t = pu.tile([OC, OWN_ROWS, W], f32, name="b_t", tag="b_t")
        nc.scalar.activation(out=a_t.rearrange("p a b -> p (a b)"), in_=pam_sb,
                             func=AF.Identity, bias=0.0, scale=0.75)
        nc.scalar.activation(out=b_t.rearrange("p a b -> p (a b)"), in_=pam_sb,
                             func=AF.Identity, bias=0.0, scale=0.25)
        sh = pu.tile([OC, OWN_ROWS, W, 2], f32, name="sh", tag="sh")
        nc.vector.tensor_copy(out=sh[:, :, 0, 0], in_=su[:, :, 0])
        nc.vector.tensor_add(sh[:, :, 1:W, 0], b_t[:, :, 0:W - 1], a_t[:, :, 1:W])
        nc.vector.tensor_add(sh[:, :, 0:W - 1, 1], a_t[:, :, 0:W - 1], b_t[:, :, 1:W])
        nc.vector.tensor_copy(out=sh[:, :, W - 1, 1], in_=su[:, :, W - 1])
        au = pu.tile([OC, OWN_ROWS, 2 * W], f32, name="au", tag="au")
        bu = pu.tile([OC, OWN_ROWS, 2 * W], f32, name="bu", tag="bu")
        shf = sh.rearrange("p a b c -> p a (b c)")
        nc.scalar.activation(out=au.rearrange("p a b -> p (a b)"),
                             in_=shf.rearrange("p a b -> p (a b)"),
                             func=AF.Identity, bias=0.0, scale=0.75)
        nc.scalar.activation(out=bu.rearrange("p a b -> p (a b)"),
                             in_=shf.rearrange("p a b -> p (a b)"),
                             func=AF.Identity, bias=0.0, scale=0.25)
        out_t = pu.tile([OC, H // 2, 2, 2 * W], f32, name="out_t", tag="out_t")
        nc.vector.tensor_copy(out=out_t[:, 0, 0, :], in_=shf[:, 0, :])
        nc.vector.tensor_add(out_t[:, 1:H // 2, 0, :], bu[:, 0:H // 2 - 1, :],
                             au[:, 1:H // 2, :])
        nc.vector.tensor_add(out_t[:, 0:H // 2, 1, :], au[:, 0:H // 2, :],
                             bu[:, 1:H // 2 + 1, :])
        nc.sync.dma_start(out=y_d, in_=out_t.rearrange("p a b c -> p (a b) c"))


SOLO = False  # timing experiment: replace collectives with local DMAs


def _build():
    nc = bacc.Bacc("TRN2", target_bir_lowering=False, debug=False,
                   enable_asserts=True, num_devices=8)

    def din(name, shape, dtp=f32):
        return nc.dram_tensor(name, shape, dtp, kind="ExternalInput").ap()

    d = {
        'xs': din("xs", [NCH, P, XR, XW], f32r),
        'wpm': din("wpm", [NCH, NT, P, P], f32r),
        'wcm': din("wcm", [NCH, NT, P, P], f32r),
        'sp': din("sp", [NCH, P, 1]), 'bp': din("bp", [NCH, P, 1]),
        'sc': din("sc", [NCH, P, 1]), 'bc': din("bc", [NCH, P, 1]),
        'wq': din("wq", [NCH, P, CR], f32r), 'wk': din("wk", [NCH, P, CR], f32r),
        'bq': din("bq", [CR, 1]), 'bk': din("bk", [CR, 1]),
        'wv': din("wv", [NCH, P, C], f32r), 'bv': din("bv", [1, C]),
        'alpha': din("alpha", [1, 1]),
        'wpo': din("wpo", [NCH, P, OC], fp16), 'bpo': din("bpo", [OC, 1]),
        'wc1': din("wc1", [NCH, P, CR]), 'bc1': din("bc1", [CR, 1]),
        'wc2': din("wc2", [NCH, CR, P]), 'bc2': din("bc2", [NCH, P, 1]),
        'wco': din("wco", [NCH, P, OC]), 'bco': din("bco", [OC, 1]),
        'ident': din("ident", [P, P], fp16),
    }
    y_d = nc.dram_tensor("y", [OC, H, 2 * W], f32, kind="ExternalOutput").ap()

    with tile.TileContext(nc) as tc:
        with tc.tile_pool(name="consts", bufs=1) as pc, \
             tc.tile_pool(name="fdram", bufs=1, space="DRAM") as pfd, \
             tc.tile_pool(name="f16_store", bufs=1) as p_f16, \
             tc.tile_pool(name="g_store", bufs=1) as p_g, \
             tc.tile_pool(name="qk_sb", bufs=1) as p_qk, \
             tc.tile_pool(name="pam_out", bufs=1) as p_pam:
            ksc = pfd.tile([CR, KOWN], f32r, name="ksc", tag="ksc")
            kg = pfd.tile([2, CR, KOWN], f32r, name="kg", tag="kg")
            vsc = pfd.tile([P, KJC, C], fp16, name="vsc", tag="vsc")
            vg = pfd.tile([2, P, KJC, C], fp16, name="vg", tag="vg")
            msc = pfd.tile([NCH, P, 1], f32, name="msc", tag="msc")
            msg = pfd.tile([NCH, P, 1], f32, name="msg", tag="msg")

            f16_t = [p_f16.tile([P, OWN], fp16, name=f"h{i}", tag=f"h{i}")
                     for i in range(NCH)]
            g_t = [p_g.tile([P, OWN], fp16, name=f"g{i}", tag=f"g{i}")
                   for i in range(NCH)]
            q_t = p_qk.tile([CR, OWN], f32r, name="q_t", tag="q_t")
            k_t = p_qk.tile([CR, HW], f32r, name="k2_t", tag="k2_t")
            pam_sb = p_pam.tile([OC, OWN], f32, name="pam_sb", tag="pam_sb")
            ct = {}

            with tc.tile_pool(name="xs_sb", bufs=1) as px:
                # x + first PAM weight chunk go out first on the ACT hwdge
                # queue; everything small rides the SP queue behind them
                x_t = [px.tile([P, XR, XW], f32r, name=f"x{i}", tag=f"x{i}")
                       for i in range(NCH)]
                for i in range(NCH):
                    nc.scalar.dma_start(out=x_t[i], in_=d['xs'][i])

                with tc.tile_pool(name="pamw", bufs=2) as pwp, \
                     tc.tile_pool(name="f_store", bufs=1) as p_f:
                    wt0 = pwp.tile([P, NT, P], f32r, tag="w", name="w")
                    nc.scalar.dma_start(
                        out=wt0, in_=d['wpm'][0].rearrange("t p f -> p t f"))

                    ct['ident'] = pc.tile([P, P], fp16, name="ident", tag="ident")
                    nc.sync.dma_start(out=ct['ident'], in_=d['ident'])
                    ct['alpha'] = pc.tile([P, 1], f32, name="alpha_t", tag="alpha_t")
                    nc.sync.dma_start(out=ct['alpha'],
                                      in_=d['alpha'].to_broadcast([P, 1]))
                    ct['bv'] = pc.tile([P, C], f32, name="bv_t", tag="bv_t")
                    nc.sync.dma_start(out=ct['bv'], in_=d['bv'].to_broadcast([P, C]))
                    for nm, rows in (('bq', CR), ('bk', CR), ('bpo', OC),
                                     ('bco', OC), ('bc1', CR)):
                        ct[nm] = pc.tile([rows, 1], f32, name=f"bias_{nm}",
                                         tag=f"bias_{nm}")
                        nc.sync.dma_start(out=ct[nm], in_=d[nm])
                    for nm in ('sp', 'bp', 'sc', 'bc'):
                        ct[nm] = [pc.tile([P, 1], f32, name=f"{nm}_{i}_t",
                                          tag=f"{nm}_{i}_t") for i in range(NCH)]
                        for i in range(NCH):
                            nc.sync.dma_start(out=ct[nm][i], in_=d[nm][i])
                    ct['wpo'] = [pc.tile([P, OC], fp16, name=f"wpo{i}_t",
                                         tag=f"wpo{i}_t") for i in range(NCH)]
                    for i in range(NCH):
                        nc.sync.dma_start(out=ct['wpo'][i], in_=d['wpo'][i])
                    # q/k/v proj weights live only until the proj phase ends
                    for nm, fr in (('wq', CR), ('wk', CR), ('wv', C)):
                        ct[nm] = [p_f.tile([P, fr], f32r, name=f"{nm}{i}_t",
                                           tag=f"{nm}{i}_t") for i in range(NCH)]
                        for i in range(NCH):
                            nc.sync.dma_start(out=ct[nm][i], in_=d[nm][i])

                    f_t = [p_f.tile([P, OWN], f32r, name=f"f{i}", tag=f"f{i}")
                           for i in range(NCH)]
                    k_own = p_f.tile([CR, KOWN], f32r, name="k_own", tag="k_own")
                    _emit_conv(nc, tc, d['wpm'], ct['sp'], ct['bp'], x_t, f_t,
                               f32, "pamconv", pwp, wt0)
                    _emit_proj(nc, tc, d, ct, f_t, f16_t, q_t, k_own, ksc, vsc)
                    for i in range(NCH):
                        nc.vector.tensor_copy(out=f16_t[i], in_=f_t[i])
                    if SOLO:
                        nc.sync.dma_start(out=kg[0], in_=ksc)
                        nc.sync.dma_start(out=kg[1], in_=ksc)
                        nc.sync.dma_start(out=vg[0], in_=vsc)
                        nc.sync.dma_start(out=vg[1], in_=vsc)
                    else:
                        nc.gpsimd.collective_compute(
                            "AllGather", OP.bypass, replica_groups=RG,
                            ins=[ksc.opt()], outs=[kg.opt()])
                        nc.gpsimd.collective_compute(
                            "AllGather", OP.bypass, replica_groups=RG,
                            ins=[vsc.opt()], outs=[vg.opt()])
                    # prefetch gathered k into SBUF while the CAM conv runs
                    nc.sync.dma_start(out=k_t[:, 0:KOWN], in_=kg[0])
                    nc.sync.dma_start(out=k_t[:, KOWN:HW], in_=kg[1])

                with tc.tile_pool(name="camw", bufs=2) as pwc:
                    wt0c = pwc.tile([P, NT, P], f32r, tag="w", name="w")
                    nc.scalar.dma_start(
                        out=wt0c, in_=d['wcm'][0].rearrange("t p f -> p t f"))
                    _emit_conv(nc, tc, d['wcm'], ct['sc'], ct['bc'], x_t, g_t,
                               fp16, "camconv", pwc, wt0c)
                    with tc.tile_pool(name="msump", bufs=1) as pms:
                        for i in range(NCH):
                            ms = pms.tile([P, 1], f32, name=f"pm{i}", tag=f"pm{i}")
                            nc.vector.tensor_reduce(
                                out=ms, in_=g_t[i][:, 0:KOWN],
                                axis=mybir.AxisListType.X, op=OP.add)
                            nc.sync.dma_start(out=msc[i], in_=ms)
                        if SOLO:
                            nc.sync.dma_start(out=msg, in_=msc)
                        else:
                            nc.gpsimd.collective_compute(
                                "AllReduce", OP.add, replica_groups=RG,
                                ins=[msc.opt()], outs=[msg.opt()])

            _emit_attention(nc, tc, ct, f16_t, q_t, k_t, pam_sb, vg)
            _emit_cam_tail(nc, tc, d, ct, g_t, pam_sb, msg)
            _emit_upsample(nc, tc, pam_sb, y_d)
    nc.compile()
    return nc


_NC_CACHE = None


def _get_nc():
    global _NC_CACHE
    if _NC_CACHE is None:
        _NC_CACHE = _build()
    return _NC_CACHE


_TAP_CI = np.array([t[0] for t in TAPS])
_TAP_DY = np.array([t[1] + 1 for t in TAPS])
_TAP_DX = np.array([t[2] + 1 for t in TAPS])


def _pack_conv(wfull):
    """[C, C, 3, 3] -> [NCH(co), NT, P(ci_local), P(co_local)] lhsT tiles."""
    wr = np.asarray(wfull, np.float32).reshape(NCH, P, NCH, P, 3, 3)
    wt = wr.transpose(0, 2, 4, 5, 3, 1)  # [co, ci, dy, dx, ci_l, co_l]
    return np.ascontiguousarray(wt[:, _TAP_CI, _TAP_DY, _TAP_DX])


def _packT(w, free):
    """w [free, C] -> [NCH, P, free] lhsT chunks."""
    return np.ascontiguousarray(np.asarray(w, np.float32).T.reshape(NCH, P, free))


def _prep_shared(inputs, flip):
    wp = np.asarray(inputs['W_pam_in'], np.float32)
    wc = np.asarray(inputs['W_cam_in'], np.float32)
    if flip:
        wp = wp[:, :, ::-1, :]
        wc = wc[:, :, ::-1, :]

    def bnfold(g, b, m, v):
        s = (np.asarray(g, np.float32)
             / np.sqrt(np.asarray(v, np.float32) + EPS)).astype(np.float32)
        bb = (np.asarray(b, np.float32)
              - np.asarray(m, np.float32) * s).astype(np.float32)
        return s.reshape(NCH, P, 1), bb.reshape(NCH, P, 1)

    sp, bp = bnfold(inputs['pam_gamma'], inputs['pam_beta'],
                    inputs['pam_mean'], inputs['pam_var'])
    sc, bc = bnfold(inputs['cam_gamma'], inputs['cam_beta'],
                    inputs['cam_mean'], inputs['cam_var'])
    wc2 = np.ascontiguousarray(
        np.asarray(inputs['Wc2'], np.float32).reshape(NCH, P, CR).transpose(0, 2, 1))
    return {
        'wpm': _pack_conv(wp),
        'wcm': _pack_conv(wc),
        'sp': sp, 'bp': bp, 'sc': sc, 'bc': bc,
        'wq': _packT(inputs['Wq'], CR),
        'wk': _packT(inputs['Wk'], CR),
        'bq': np.asarray(inputs['bq'], np.float32).reshape(CR, 1),
        'bk': np.asarray(inputs['bk'], np.float32).reshape(CR, 1),
        'wv': _packT(inputs['Wv'], C),
        'bv': np.asarray(inputs['bv'], np.float32).reshape(1, C),
        'alpha': np.asarray(inputs['alpha'], np.float32).reshape(1, 1),
        'wpo': _packT(inputs['W_pam_out'], OC).astype(np.float16),
        'bpo': np.asarray(inputs['b_pam_out'], np.float32).reshape(OC, 1),
        'wc1': _packT(np.asarray(inputs['Wc1'], np.float32) / HW, CR),
        'bc1': np.asarray(inputs['bc1'], np.float32).reshape(CR, 1),
        'wc2': wc2,
        'bc2': np.asarray(inputs['bc2'], np.float32).reshape(NCH, P, 1),
        'wco': _packT(inputs['W_cam_out'], OC),
        'bco': np.asarray(inputs['b_cam_out'], np.float32).reshape(OC, 1),
        'ident': np.eye(P, dtype=np.float16),
    }


def _make_in_maps(inputs):
    x = np.asarray(inputs['x'], np.float32)  # [4, 512, 64, 64]
    shared = {f: _prep_shared(inputs, f) for f in (False, True)}
    in_maps = []
    for c in range(8):
        s, flip = c // 2, c % 2
        xs = x[s]
        if flip:
            xs = xs[:, ::-1, :]
        xp = np.zeros((C, XR, XW), np.float32)
        xp[:, :, 1:1 + W] = xs[:, 0:XR, :]
        m = dict(shared[bool(flip)])
        m['xs'] = np.ascontiguousarray(xp.reshape(NCH, P, XR, XW))
        in_maps.append(m)
    return in_maps


def kernel(**inputs):
    nc = _get_nc()
    in_maps = _make_in_maps(inputs)
    res = run_bass_kernel_spmd(nc, in_maps, list(range(8)))
    out = np.empty((4, OC, 2 * H, 2 * W), np.float32)
    for c in range(8):
        s, flip = c // 2, c % 2
        o = res.results[c]['y']  # [64, 64, 128]
        if flip:
            out[s, :, H:2 * H, :] = o[:, ::-1, :]
        else:
            out[s, :, 0:H, :] = o
    return out


# revision 25
# speedup vs baseline: 3.2061x; 1.2257x over previous
"""DAHead (dual-attention head) Trainium2 kernel, v2.

8-core SPMD with pair collectives: core c handles sample c//2, spatial
half c%2. Odd cores see the sample vertically flipped (conv weights
flipped along dy) so every core runs the same program; the host
un-flips the output half.

v2 vs v1: the conv3x3 / q/k/v work is split across the sample's core
PAIR: each core computes f and g only on its own 34 rows, k/v only on
its own 32 rows, and the pair AllGathers k (f32, 0.5MB) and vT (fp16,
2MB) plus AllReduces the CAM channel-mean, overlapped with the CAM
conv. Attention j-order becomes [pair-rank0 rows, pair-rank1 rows],
a permutation of the 4096 tokens - softmax+apply are
permutation-invariant over j.

Precision: the softmax is argmax-like (logits up to ~375), so the
q/k path runs in f32r (full-rate PE mode, ~1.6e-4 matmul error vs
fp16's 2.9e-4): PAM conv f32r -> f kept f32 in SBUF -> q/k f32r ->
logits f32r. The linear paths (v, attention apply, residual,
out-projs, whole CAM branch) are fp16/f32r single-pass.

Per-core program:
  1. conv3x3+BN+lrelu (PAM) f32r, rows 0..33 -> f f32 in SBUF.
  2. k (rows 0..31) f32, vT (rows 0..31) fp16, q (rows 0..33) f32;
     k/vT -> DRAM -> pair AllGather.
  3. conv3x3+BN+lrelu (CAM) f32r rows 0..33 -> g fp16 in SBUF;
     partial channel sums over rows 0..31 -> pair AllReduce.
  4. Attention over own 17 i-chunks: f32r logits, row softmax (max on
     gpsimd, exp+scale on ACT), PE transpose of prob chunks, fp16
     apply, residual fused to fp16, 1x1 out-proj -> pam_sb f32.
  5. CAM MLP (sigmoid folded into the 1x1 weights), out-proj
     accumulated into pam_sb.
  6. bilinear x2 upsample (DVE) of rows 0..32, DMA out.
"""
import sys

if '/opt/trn_rl_repo' not in sys.path:
    sys.path.insert(0, '/opt/trn_rl_repo')

import numpy as np

import concourse.bass as bass
import concourse.mybir as mybir
import concourse.tile as tile
from concourse import bacc
from concourse.bass_utils import run_bass_kernel_spmd

dt = mybir.dt
f32 = dt.float32
f32r = dt.float32r
fp16 = dt.float16
AF = mybir.ActivationFunctionType
OP = mybir.AluOpType

C = 512          # channels
P = 128          # partition size
NCH = C // P     # channel chunks (4)
H = W = 64
HW = H * W       # 4096
CR = 64          # q/k channels
OC = 64          # output channels
OWN_ROWS = 34    # rows computed per core (0..33); rows 32+ feed upsample only
OWN = OWN_ROWS * W    # 2176 = 17*128
NIC = OWN // P        # 17 attention i-chunks
KV_ROWS = 32          # rows contributed to the gathered k/v (0..31)
KOWN = KV_ROWS * W    # 2048
KJC = KOWN // P       # 16 own j-chunks
NJC = HW // P         # 32 gathered j-chunks
XR = OWN_ROWS + 1     # x rows needed (0..34: +1 halo row below)
XW = W + 2            # x cols incl zero-pad columns (f32r needs even APs)
EPS = 1e-5
RG = [[0, 1], [2, 3], [4, 5], [6, 7]]   # sample pairs

# conv h-blocks (start_row, n_rows); 7-row blocks keep the moving dim
# >= 256 even with dx-edge clamps (f32r full-rate needs >= 256)
HBLK = [(0, 7), (7, 7), (14, 7), (21, 7), (28, 6)]

# tap order: full-coverage center tap first (needed for PSUM start flag)
_ALL = [(ci, dy, dx) for ci in range(NCH) for dy in (-1, 0, 1) for dx in (-1, 0, 1)]
TAPS = [(0, 0, 0)] + [t for t in _ALL if t != (0, 0, 0)]
NT = len(TAPS)   # 36

Q_EDGES = [0, 512, 1024, 1536, 2048, OWN]
K_EDGES = [0, 512, 1024, 1536, 2048]

# attention i-blocks (first_chunk, n_chunks)
IBLK = [(0, 4), (4, 4), (8, 4), (12, 4), (16, 1)]


def _conv_tap_aps(psum_t, x_t, r0b, nr, dy, dx):
    """APs for one conv tap on block rows [r0b, r0b+nr). x_t: [128,XR,XW]
    with zero-padded columns 0 and 65 (f32r needs even-width APs)."""
    r0 = 1 if (r0b == 0 and dy == -1) else 0
    out_ap = psum_t[:, r0:nr, :]
    in_ap = x_t[:, r0b + r0 + dy: r0b + nr + dy, 1 + dx: 1 + dx + W]
    return out_ap, in_ap


def _emit_conv(nc, tc, w_d, s_ct, b_ct, x_t, dst, odt, pool_name, pw, wt0):
    """conv3x3+BN+lrelu f32r over rows 0..33 -> dst[co] [P, OWN] (odt)."""
    with tc.tile_pool(name=pool_name + "_ev", bufs=3) as pe, \
         tc.tile_pool(name=pool_name + "_ps", bufs=1, space="PSUM") as psc:
        for co in range(NCH):
            if co == 0:
                wt = wt0
            else:
                wt = pw.tile([P, NT, P], f32r, tag="w", name="w")
                nc.scalar.dma_start(out=wt, in_=w_d[co].rearrange("t p f -> p t f"))
            for b, (r0b, nr) in enumerate(HBLK):
                pst = psc.tile([P, nr, W], f32, tag="cv", name="cv")
                for t, (ci, dy, dx) in enumerate(TAPS):
                    o_ap, i_ap = _conv_tap_aps(pst, x_t[ci], r0b, nr, dy, dx)
                    nc.tensor.matmul(o_ap, wt[:, t, :], i_ap,
                                     start=(t == 0), stop=(t == NT - 1))
                z = pe.tile([P, 7 * W], odt, tag="z", name="z")[:, 0:nr * W]
                nc.scalar.activation(
                    out=z, in_=pst.rearrange("p a b -> p (a b)"),
                    func=AF.Identity, bias=b_ct[co], scale=s_ct[co])
                nc.vector.scalar_tensor_tensor(
                    out=dst[co][:, r0b * W:(r0b + nr) * W], in0=z, scalar=0.2,
                    in1=z, op0=OP.mult, op1=OP.max)


def _emit_proj(nc, tc, d, ct, f_t, f16_t, q_t, k_own, ksc, vsc):
    """k/v own-row projections -> DRAM (gathered later), q -> SBUF."""
    with tc.tile_pool(name="proj_ev", bufs=2) as pve, \
         tc.tile_pool(name="proj_ps", bufs=2, space="PSUM") as psq:
        # k first so its AllGather starts earliest
        for bi in range(len(K_EDGES) - 1):
            off, end = K_EDGES[bi], K_EDGES[bi + 1]
            pq = psq.tile([CR, 512], f32, tag="pq", name="pq")
            for ci in range(NCH):
                nc.tensor.matmul(pq, ct['wk'][ci], f_t[ci][:, off:end],
                                 start=(ci == 0), stop=(ci == NCH - 1))
            nc.scalar.activation(out=k_own[:, off:end], in_=pq,
                                 func=AF.Identity, bias=ct['bk'], scale=1.0)
        nc.sync.dma_start(out=ksc, in_=k_own)

        for jc in range(KJC):
            pv = psq.tile([P, C], f32, tag="pv", name="pv")
            s, e = jc * P, (jc + 1) * P
            for ci in range(NCH):
                nc.tensor.matmul(pv, f_t[ci][:, s:e], ct['wv'][ci],
                                 start=(ci == 0), stop=(ci == NCH - 1))
            vtmp = pve.tile([P, C], fp16, tag="vtmp", name="vtmp")
            nc.vector.tensor_add(vtmp, pv, ct['bv'])
            nc.sync.dma_start(out=vsc[:, jc, :], in_=vtmp)

        for bi in range(len(Q_EDGES) - 1):
            off, end = Q_EDGES[bi], Q_EDGES[bi + 1]
            pq = psq.tile([CR, 512], f32, tag="pq", name="pq")[:, 0:end - off]
            for ci in range(NCH):
                nc.tensor.matmul(pq, ct['wq'][ci], f_t[ci][:, off:end],
                                 start=(ci == 0), stop=(ci == NCH - 1))
            nc.scalar.activation(out=q_t[:, off:end], in_=pq,
                                 func=AF.Identity, bias=ct['bq'], scale=1.0)


def _emit_attention(nc, tc, ct, f_t, q_t, k_t, pam_sb, vg):
    with tc.tile_pool(name="kv2", bufs=1) as pv2, \
         tc.tile_pool(name="ls", bufs=2) as pls, \
         tc.tile_pool(name="e16p", bufs=2) as pep, \
         tc.tile_pool(name="et", bufs=1) as pet, \
         tc.tile_pool(name="att_tmp", bufs=2) as pat, \
         tc.tile_pool(name="res_t", bufs=2) as prs, \
         tc.tile_pool(name="ps_l", bufs=2, space="PSUM") as psl, \
         tc.tile_pool(name="ps_t", bufs=2, space="PSUM") as pstp, \
         tc.tile_pool(name="ps_a", bufs=2, space="PSUM") as psa, \
         tc.tile_pool(name="ps_p", bufs=2, space="PSUM") as psp:
        vt_t = pv2.tile([P, NJC, C], fp16, name="vt2_t", tag="vt2_t")
        nc.sync.dma_start(out=vt_t[:, 0:KJC, :], in_=vg[0])
        nc.sync.dma_start(out=vt_t[:, KJC:NJC, :], in_=vg[1])

        def emit_softmax(ic):
            """logits + row softmax for i-chunk ic -> e16 (fp16 probs)."""
            ls = pls.tile([P, HW], f32, tag="ls", name="ls")
            for jb in range(HW // 512):
                pl = psl.tile([P, 512], f32, tag="pl", name="pl")
                nc.tensor.matmul(
                    pl, q_t[:, ic * P:(ic + 1) * P],
                    k_t[:, jb * 512:(jb + 1) * 512], start=True, stop=True)
                nc.scalar.activation(
                    out=ls[:, jb * 512:(jb + 1) * 512], in_=pl,
                    func=AF.Identity, bias=0.0, scale=1.0)
            nmax = pat.tile([P, 1], f32, tag="nmax", name="nmax")
            nc.vector.tensor_reduce(out=nmax, in_=ls, axis=mybir.AxisListType.X,
                                    op=OP.max, negate=True)
            rsum = pat.tile([P, 1], f32, tag="rsum", name="rsum")
            nc.scalar.activation(out=ls, in_=ls, func=AF.Exp,
                                 bias=nmax, scale=1.0, accum_out=rsum)
            rrec = pat.tile([P, 1], f32, tag="rrec", name="rrec")
            nc.vector.reciprocal(out=rrec, in_=rsum)
            e16 = pep.tile([P, HW], fp16, tag="e16", name="e16")
            nc.scalar.activation(out=e16, in_=ls, func=AF.Identity,
                                 bias=0.0, scale=rrec)
            return e16

        def emit_transposes(e16, et_t, ph):
            for jc in range(NJC):
                pt = pstp.tile([P, P], fp16, tag="pt", name="pt")
                nc.tensor.transpose(
                    pt, e16[:, jc * P:(jc + 1) * P], ct['ident'])
                if jc % 2 == 0:
                    nc.vector.tensor_copy(
                        out=et_t[:, jc, ph * P:(ph + 1) * P], in_=pt)
                else:
                    nc.scalar.activation(
                        out=et_t[:, jc, ph * P:(ph + 1) * P], in_=pt,
                        func=AF.Identity, bias=0.0, scale=1.0)

        # software pipeline: block b's apply covers block b+1's softmax chain
        e16s = {ic: emit_softmax(ic) for ic in (0, 1, 2, 3)}
        for ib, (ic0, nch) in enumerate(IBLK):
            isz = P * nch
            ioff = ic0 * P
            et_t = pet.tile([P, NJC, 4 * P], fp16, tag="et", name="et")
            for ph in range(nch):
                emit_transposes(e16s.pop(ic0 + ph), et_t, ph)
            if ib + 1 < len(IBLK):
                nic0, nnch = IBLK[ib + 1]
                for ph in range(nnch):
                    e16s[nic0 + ph] = emit_softmax(nic0 + ph)
            r16 = []
            for co in range(NCH):
                pa = psa.tile([P, 4 * P], f32, tag="pa", name="pa")[:, 0:isz]
                for jc in range(NJC):
                    nc.tensor.matmul(
                        pa, vt_t[:, jc, co * P:(co + 1) * P],
                        et_t[:, jc, 0:isz],
                        start=(jc == 0), stop=(jc == NJC - 1))
                rt = prs.tile([P, 4 * P], fp16, tag=f"r{co}", name=f"r{co}")[:, 0:isz]
                nc.vector.scalar_tensor_tensor(
                    out=rt, in0=pa, scalar=ct['alpha'],
                    in1=f_t[co][:, ioff:ioff + isz], op0=OP.mult, op1=OP.add)
                r16.append(rt)
            pp = psp.tile([OC, 4 * P], f32, tag="pp", name="pp")[:, 0:isz]
            for ci in range(NCH):
                nc.tensor.matmul(pp, ct['wpo'][ci], r16[ci],
                                 start=(ci == 0), stop=(ci == NCH - 1))
            nc.scalar.activation(out=pam_sb[:, ioff:ioff + isz], in_=pp,
                                 func=AF.Identity, bias=ct['bpo'], scale=1.0)


def _emit_cam_tail(nc, tc, d, ct, g_t, pam_sb, msg):
    """channel-attention MLP (mean already AllReduced into msg) + 1x1 out."""
    with tc.tile_pool(name="mlp", bufs=1) as pm, \
         tc.tile_pool(name="cam_ev", bufs=2) as pce, \
         tc.tile_pool(name="ps_mlp", bufs=2, space="PSUM") as psm, \
         tc.tile_pool(name="ps_co", bufs=2, space="PSUM") as psco:
        msum = [pm.tile([P, 1], f32, name=f"ms{i}", tag=f"ms{i}") for i in range(NCH)]
        for i in range(NCH):
            nc.sync.dma_start(out=msum[i], in_=msg[i])
        wc1_t = [pm.tile([P, CR], f32, name=f"w1{i}", tag=f"w1{i}") for i in range(NCH)]
        wc2_t = [pm.tile([CR, P], f32, name=f"w2{i}", tag=f"w2{i}") for i in range(NCH)]
        wco_t = [pm.tile([P, OC], f32, name=f"wo{i}", tag=f"wo{i}") for i in range(NCH)]
        bc2_t = [pm.tile([P, 1], f32, name=f"b2{i}", tag=f"b2{i}") for i in range(NCH)]
        for i in range(NCH):
            nc.sync.dma_start(out=wc1_t[i], in_=d['wc1'][i])
            nc.sync.dma_start(out=wc2_t[i], in_=d['wc2'][i])
            nc.sync.dma_start(out=wco_t[i], in_=d['wco'][i])
            nc.sync.dma_start(out=bc2_t[i], in_=d['bc2'][i])
        p1 = psm.tile([CR, 1], f32, tag="p1", name="p1")
        for ci in range(NCH):
            nc.tensor.matmul(p1, wc1_t[ci], msum[ci],
                             start=(ci == 0), stop=(ci == NCH - 1))
        t1 = pm.tile([CR, 1], f32, name="t1", tag="t1")
        nc.scalar.activation(out=t1, in_=p1, func=AF.Identity,
                             bias=ct['bc1'], scale=1.0)
        y1 = pm.tile([CR, 1], f32, name="y1", tag="y1")
        nc.vector.scalar_tensor_tensor(out=y1, in0=t1, scalar=0.2, in1=t1,
                                       op0=OP.mult, op1=OP.max)
        wce = [pm.tile([P, OC], fp16, name=f"we{i}", tag=f"we{i}") for i in range(NCH)]
        for co in range(NCH):
            p2 = psm.tile([P, 1], f32, tag="p2", name="p2")
            nc.tensor.matmul(p2, wc2_t[co], y1, start=True, stop=True)
            s_t = pm.tile([P, 1], f32, name=f"s{co}", tag=f"s{co}")
            nc.scalar.activation(out=s_t, in_=p2, func=AF.Sigmoid,
                                 bias=bc2_t[co], scale=1.0)
            nc.vector.tensor_scalar_mul(wce[co], wco_t[co], s_t)
        for bi in range(len(Q_EDGES) - 1):
            off, end = Q_EDGES[bi], Q_EDGES[bi + 1]
            sz = end - off
            pco = psco.tile([OC, 512], f32, tag="pco", name="pco")[:, 0:sz]
            for ci in range(NCH):
                nc.tensor.matmul(pco, wce[ci], g_t[ci][:, off:end],
                                 start=(ci == 0), stop=(ci == NCH - 1))
            zc = pce.tile([OC, 512], f32, tag="zc", name="zc")[:, 0:sz]
            nc.scalar.activation(out=zc, in_=pco, func=AF.Identity,
                                 bias=ct['bco'], scale=1.0)
            nc.vector.tensor_add(pam_sb[:, off:end], pam_sb[:, off:end], zc)


def _emit_upsample(nc, tc, pam_sb, y_d):
    with tc.tile_pool(name="up", bufs=1) as pu:
        su = pam_sb.rearrange("p (a b) -> p a b", b=W)  # [OC,34,64]
        a_t = pu.tile([OC, OWN_ROWS, W], f32, name="a_t", tag="a_t")
        b_t = pu.tile([OC, OWN_ROWS, W], f32, name="b_t", tag="b_t")
        nc.scalar.activation(out=a_t.rearrange("p a b -> p (a b)"), in_=pam_sb,
                             func=AF.Identity, bias=0.0, scale=0.75)
        nc.scalar.activation(out=b_t.rearrange("p a b -> p (a b)"), in_=pam_sb,
                             func=AF.Identity, bias=0.0, scale=0.25)
        sh = pu.tile([OC, OWN_ROWS, W, 2], f32, name="sh", tag="sh")
        nc.vector.tensor_copy(out=sh[:, :, 0, 0], in_=su[:, :, 0])
        nc.vector.tensor_add(sh[:, :, 1:W, 0], b_t[:, :, 0:W - 1], a_t[:, :, 1:W])
        nc.vector.tensor_add(sh[:, :, 0:W - 1, 1], a_t[:, :, 0:W - 1], b_t[:, :, 1:W])
        nc.vector.tensor_copy(out=sh[:, :, W - 1, 1], in_=su[:, :, W - 1])
        au = pu.tile([OC, OWN_ROWS, 2 * W], f32, name="au", tag="au")
        bu = pu.tile([OC, OWN_ROWS, 2 * W], f32, name="bu", tag="bu")
        shf = sh.rearrange("p a b c -> p a (b c)")
        nc.scalar.activation(out=au.rearrange("p a b -> p (a b)"),
                             in_=shf.rearrange("p a b -> p (a b)"),
                             func=AF.Identity, bias=0.0, scale=0.75)
        nc.scalar.activation(out=bu.rearrange("p a b -> p (a b)"),
                             in_=shf.rearrange("p a b -> p (a b)"),
                             func=AF.Identity, bias=0.0, scale=0.25)
        out_t = pu.tile([OC, H // 2, 2, 2 * W], f32, name="out_t", tag="out_t")
        nc.vector.tensor_copy(out=out_t[:, 0, 0, :], in_=shf[:, 0, :])
        nc.vector.tensor_add(out_t[:, 1:H // 2, 0, :], bu[:, 0:H // 2 - 1, :],
                             au[:, 1:H // 2, :])
        nc.vector.tensor_add(out_t[:, 0:H // 2, 1, :], au[:, 0:H // 2, :],
                             bu[:, 1:H // 2 + 1, :])
        nc.sync.dma_start(out=y_d, in_=out_t.rearrange("p a b c -> p (a b) c"))


SOLO = False  # timing experiment: replace collectives with local DMAs


def _build():
    nc = bacc.Bacc("TRN2", target_bir_lowering=False, debug=False,
                   enable_asserts=True, num_devices=8)

    def din(name, shape, dtp=f32):
        return nc.dram_tensor(name, shape, dtp, kind="ExternalInput").ap()

    d = {
        'xs': din("xs", [NCH, P, XR, XW], f32r),
        'wpm': din("wpm", [NCH, NT, P, P], f32r),
        'wcm': din("wcm", [NCH, NT, P, P], f32r),
        'sp': din("sp", [NCH, P, 1]), 'bp': din("bp", [NCH, P, 1]),
        'sc': din("sc", [NCH, P, 1]), 'bc': din("bc", [NCH, P, 1]),
        'wq': din("wq", [NCH, P, CR], f32r), 'wk': din("wk", [NCH, P, CR], f32r),
        'bq': din("bq", [CR, 1]), 'bk': din("bk", [CR, 1]),
        'wv': din("wv", [NCH, P, C], f32r), 'bv': din("bv", [1, C]),
        'alpha': din("alpha", [1, 1]),
        'wpo': din("wpo", [NCH, P, OC], fp16), 'bpo': din("bpo", [OC, 1]),
        'wc1': din("wc1", [NCH, P, CR]), 'bc1': din("bc1", [CR, 1]),
        'wc2': din("wc2", [NCH, CR, P]), 'bc2': din("bc2", [NCH, P, 1]),
        'wco': din("wco", [NCH, P, OC]), 'bco': din("bco", [OC, 1]),
        'ident': din("ident", [P, P], fp16),
    }
    y_d = nc.dram_tensor("y", [OC, H, 2 * W], f32, kind="ExternalOutput").ap()

    with tile.TileContext(nc) as tc:
        with tc.tile_pool(name="consts", bufs=1) as pc, \
             tc.tile_pool(name="fdram", bufs=1, space="DRAM") as pfd, \
             tc.tile_pool(name="f16_store", bufs=1) as p_f16, \
             tc.tile_pool(name="g_store", bufs=1) as p_g, \
             tc.tile_pool(name="qk_sb", bufs=1) as p_qk, \
             tc.tile_pool(name="pam_out", bufs=1) as p_pam:
            ksc = pfd.tile([CR, KOWN], f32r, name="ksc", tag="ksc")
            kg = pfd.tile([2, CR, KOWN], f32r, name="kg", tag="kg")
            vsc = pfd.tile([P, KJC, C], fp16, name="vsc", tag="vsc")
            vg = pfd.tile([2, P, KJC, C], fp16, name="vg", tag="vg")
            msc = pfd.tile([NCH, P, 1], f32, name="msc", tag="msc")
            msg = pfd.tile([NCH, P, 1], f32, name="msg", tag="msg")

            f16_t = [p_f16.tile([P, OWN], fp16, name=f"h{i}", tag=f"h{i}")
                     for i in range(NCH)]
            g_t = [p_g.tile([P, OWN], fp16, name=f"g{i}", tag=f"g{i}")
                   for i in range(NCH)]
            q_t = p_qk.tile([CR, OWN], f32r, name="q_t", tag="q_t")
            k_t = p_qk.tile([CR, HW], f32r, name="k2_t", tag="k2_t")
            pam_sb = p_pam.tile([OC, OWN], f32, name="pam_sb", tag="pam_sb")
            ct = {}

            with tc.tile_pool(name="xs_sb", bufs=1) as px:
                # x + first PAM weight chunk go out first on the ACT hwdge
                # queue; everything small rides the SP queue behind them
                x_t = [px.tile([P, XR, XW], f32r, name=f"x{i}", tag=f"x{i}")
                       for i in range(NCH)]
                for i in range(NCH):
                    nc.scalar.dma_start(out=x_t[i], in_=d['xs'][i])

                with tc.tile_pool(name="pamw", bufs=2) as pwp, \
                     tc.tile_pool(name="f_store", bufs=1) as p_f:
                    wt0 = pwp.tile([P, NT, P], f32r, tag="w", name="w")
                    nc.scalar.dma_start(
                        out=wt0, in_=d['wpm'][0].rearrange("t p f -> p t f"))

                    ct['ident'] = pc.tile([P, P], fp16, name="ident", tag="ident")
                    nc.sync.dma_start(out=ct['ident'], in_=d['ident'])
                    ct['alpha'] = pc.tile([P, 1], f32, name="alpha_t", tag="alpha_t")
                    nc.sync.dma_start(out=ct['alpha'],
                                      in_=d['alpha'].to_broadcast([P, 1]))
                    ct['bv'] = pc.tile([P, C], f32, name="bv_t", tag="bv_t")
                    nc.sync.dma_start(out=ct['bv'], in_=d['bv'].to_broadcast([P, C]))
                    for nm, rows in (('bq', CR), ('bk', CR), ('bpo', OC),
                                     ('bco', OC), ('bc1', CR)):
                        ct[nm] = pc.tile([rows, 1], f32, name=f"bias_{nm}",
                                         tag=f"bias_{nm}")
                        nc.sync.dma_start(out=ct[nm], in_=d[nm])
                    for nm in ('sp', 'bp', 'sc', 'bc'):
                        ct[nm] = [pc.tile([P, 1], f32, name=f"{nm}_{i}_t",
                                          tag=f"{nm}_{i}_t") for i in range(NCH)]
                        for i in range(NCH):
                            nc.sync.dma_start(out=ct[nm][i], in_=d[nm][i])
                    ct['wpo'] = [pc.tile([P, OC], fp16, name=f"wpo{i}_t",
                                         tag=f"wpo{i}_t") for i in range(NCH)]
                    for i in range(NCH):
                        nc.sync.dma_start(out=ct['wpo'][i], in_=d['wpo'][i])
                    # q/k/v proj weights live only until the proj phase ends
                    for nm, fr in (('wq', CR), ('wk', CR), ('wv', C)):
                        ct[nm] = [p_f.tile([P, fr], f32r, name=f"{nm}{i}_t",
                                           tag=f"{nm}{i}_t") for i in range(NCH)]
                        for i in range(NCH):
                            nc.sync.dma_start(out=ct[nm][i], in_=d[nm][i])

                    f_t = [p_f.tile([P, OWN], f32r, name=f"f{i}", tag=f"f{i}")
                           for i in range(NCH)]
                    k_own = p_f.tile([CR, KOWN], f32r, name="k_own", tag="k_own")
                    _emit_conv(nc, tc, d['wpm'], ct['sp'], ct['bp'], x_t, f_t,
                               f32, "pamconv", pwp, wt0)
                    _emit_proj(nc, tc, d, ct, f_t, f16_t, q_t, k_own, ksc, vsc)
                    for i in range(NCH):
                        nc.vector.tensor_copy(out=f16_t[i], in_=f_t[i])
                    if SOLO:
                        nc.sync.dma_start(out=kg[0], in_=ksc)
                        nc.sync.dma_start(out=kg[1], in_=ksc)
                        nc.sync.dma_start(out=vg[0], in_=vsc)
                        nc.sync.dma_start(out=vg[1], in_=vsc)
                    else:
                        nc.gpsimd.collective_compute(
                            "AllGather", OP.bypass, replica_groups=RG,
                            ins=[ksc.opt()], outs=[kg.opt()])
                        nc.gpsimd.collective_compute(
                            "AllGather", OP.bypass, replica_groups=RG,
                            ins=[vsc.opt()], outs=[vg.opt()])
                    # prefetch gathered k into SBUF while the CAM conv runs
                    nc.sync.dma_start(out=k_t[:, 0:KOWN], in_=kg[0])
                    nc.sync.dma_start(out=k_t[:, KOWN:HW], in_=kg[1])

                with tc.tile_pool(name="camw", bufs=2) as pwc:
                    wt0c = pwc.tile([P, NT, P], f32r, tag="w", name="w")
                    nc.scalar.dma_start(
                        out=wt0c, in_=d['wcm'][0].rearrange("t p f -> p t f"))
                    _emit_conv(nc, tc, d['wcm'], ct['sc'], ct['bc'], x_t, g_t,
                               fp16, "camconv", pwc, wt0c)
                    with tc.tile_pool(name="msump", bufs=1) as pms:
                        for i in range(NCH):
                            ms = pms.tile([P, 1], f32, name=f"pm{i}", tag=f"pm{i}")
                            nc.vector.tensor_reduce(
                                out=ms, in_=g_t[i][:, 0:KOWN],
                                axis=mybir.AxisListType.X, op=OP.add)
                            nc.sync.dma_start(out=msc[i], in_=ms)
                        if SOLO:
                            nc.sync.dma_start(out=msg, in_=msc)
                        else:
                            nc.gpsimd.collective_compute(
                                "AllReduce", OP.add, replica_groups=RG,
                                ins=[msc.opt()], outs=[msg.opt()])

            _emit_attention(nc, tc, ct, f16_t, q_t, k_t, pam_sb, vg)
            _emit_cam_tail(nc, tc, d, ct, g_t, pam_sb, msg)
            _emit_upsample(nc, tc, pam_sb, y_d)
    nc.compile()
    return nc


_NC_CACHE = None


def _get_nc():
    global _NC_CACHE
    if _NC_CACHE is None:
        _NC_CACHE = _build()
    return _NC_CACHE


_TAP_CI = np.array([t[0] for t in TAPS])
_TAP_DY = np.array([t[1] + 1 for t in TAPS])
_TAP_DX = np.array([t[2] + 1 for t in TAPS])


def _pack_conv(wfull):
    """[C, C, 3, 3] -> [NCH(co), NT, P(ci_local), P(co_local)] lhsT tiles."""
    wr = np.asarray(wfull, np.float32).reshape(NCH, P, NCH, P, 3, 3)
    wt = wr.transpose(0, 2, 4, 5, 3, 1)  # [co, ci, dy, dx, ci_l, co_l]
    return np.ascontiguousarray(wt[:, _TAP_CI, _TAP_DY, _TAP_DX])


def _packT(w, free):
    """w [free, C] -> [NCH, P, free] lhsT chunks."""
    return np.ascontiguousarray(np.asarray(w, np.float32).T.reshape(NCH, P, free))


def _prep_shared(inputs, flip):
    wp = np.asarray(inputs['W_pam_in'], np.float32)
    wc = np.asarray(inputs['W_cam_in'], np.float32)
    if flip:
        wp = wp[:, :, ::-1, :]
        wc = wc[:, :, ::-1, :]

    def bnfold(g, b, m, v):
        s = (np.asarray(g, np.float32)
             / np.sqrt(np.asarray(v, np.float32) + EPS)).astype(np.float32)
        bb = (np.asarray(b, np.float32)
              - np.asarray(m, np.float32) * s).astype(np.float32)
        return s.reshape(NCH, P, 1), bb.reshape(NCH, P, 1)

    sp, bp = bnfold(inputs['pam_gamma'], inputs['pam_beta'],
                    inputs['pam_mean'], inputs['pam_var'])
    sc, bc = bnfold(inputs['cam_gamma'], inputs['cam_beta'],
                    inputs['cam_mean'], inputs['cam_var'])
    wc2 = np.ascontiguousarray(
        np.asarray(inputs['Wc2'], np.float32).reshape(NCH, P, CR).transpose(0, 2, 1))
    return {
        'wpm': _pack_conv(wp),
        'wcm': _pack_conv(wc),
        'sp': sp, 'bp': bp, 'sc': sc, 'bc': bc,
        'wq': _packT(inputs['Wq'], CR),
        'wk': _packT(inputs['Wk'], CR),
        'bq': np.asarray(inputs['bq'], np.float32).reshape(CR, 1),
        'bk': np.asarray(inputs['bk'], np.float32).reshape(CR, 1),
        'wv': _packT(inputs['Wv'], C),
        'bv': np.asarray(inputs['bv'], np.float32).reshape(1, C),
        'alpha': np.asarray(inputs['alpha'], np.float32).reshape(1, 1),
        'wpo': _packT(inputs['W_pam_out'], OC).astype(np.float16),
        'bpo': np.asarray(inputs['b_pam_out'], np.float32).reshape(OC, 1),
        'wc1': _packT(np.asarray(inputs['Wc1'], np.float32) / HW, CR),
        'bc1': np.asarray(inputs['bc1'], np.float32).reshape(CR, 1),
        'wc2': wc2,
        'bc2': np.asarray(inputs['bc2'], np.float32).reshape(NCH, P, 1),
        'wco': _packT(inputs['W_cam_out'], OC),
        'bco': np.asarray(inputs['b_cam_out'], np.float32).reshape(OC, 1),
        'ident': np.eye(P, dtype=np.float16),
    }


def _make_in_maps(inputs):
    x = np.asarray(inputs['x'], np.float32)  # [4, 512, 64, 64]
    shared = {f: _prep_shared(inputs, f) for f in (False, True)}
    in_maps = []
    for c in range(8):
        s, flip = c // 2, c % 2
        xs = x[s]
        if flip:
            xs = xs[:, ::-1, :]
        xp = np.zeros((C, XR, XW), np.float32)
        xp[:, :, 1:1 + W] = xs[:, 0:XR, :]
        m = dict(shared[bool(flip)])
        m['xs'] = np.ascontiguousarray(xp.reshape(NCH, P, XR, XW))
        in_maps.append(m)
    return in_maps


def kernel(**inputs):
    nc = _get_nc()
    in_maps = _make_in_maps(inputs)
    res = run_bass_kernel_spmd(nc, in_maps, list(range(8)))
    out = np.empty((4, OC, 2 * H, 2 * W), np.float32)
    for c in range(8):
        s, flip = c // 2, c % 2
        o = res.results[c]['y']  # [64, 64, 128]
        if flip:
            out[s, :, H:2 * H, :] = o[:, ::-1, :]
        else:
            out[s, :, 0:H, :] = o
    return out
